# revision 1
# baseline (speedup 1.0000x reference)
"""3-layer GAT (PyG GATConv semantics) forward on 8 Trainium2 NeuronCores.

Strategy (graph/data parallel, dst-sharded):
  - Nodes padded to N_PAD = 8*98*128 and sharded by destination across 8 cores.
  - Edges (plus self-loops) bucketed host-side by (core, dst-tile, src-bank),
    sorted by dst, padded to 128-edge chunks; chunk structure equalized across
    cores so one SPMD program serves all 8.
  - Per layer: each core computes x_aug = h @ [W | W*a_src | W*a_dst] for its
    node shard (x in bf16, attention logits in f32), AllGathers the packed
    row table, then processes its dst tiles: dma_gather (4 SWDGE queues)
    fetches x_aug rows by src, attention weights are computed with the
    exp(leaky_relu(al_src + al_dst)) folded per edge, and the per-dst softmax
    numerator/denominator are accumulated with one-hot matmuls on TensorE.
    al_dst is expanded dst->edges with a transposed one-hot matmul (hi/lo bf16
    split keeps f32 precision).
"""
import os
import numpy as np
import ml_dtypes

import concourse.bass as bass
import concourse.bacc as bacc
import concourse.tile as tile
import concourse.mybir as mybir
from concourse import ap_utils
from concourse.bass_utils import run_bass_kernel_spmd

F32 = mybir.dt.float32
BF16 = mybir.dt.bfloat16
U16 = mybir.dt.uint16
I16 = mybir.dt.int16
AF = mybir.ActivationFunctionType
OP = mybir.AluOpType
P = 128
BF_NP = ml_dtypes.bfloat16

LAST_RESULT = {}


# ----------------------------------------------------------------------------
# configuration
# ----------------------------------------------------------------------------
class Cfg:
    def __init__(self, n_nodes=100000, tiles_per_core=98, bank_rows=32768,
                 ncores=8, heads=(8, 8, 1), ch=(32, 32, 40), fin0=128):
        self.n_nodes = n_nodes
        self.ncores = ncores
        self.tiles = tiles_per_core
        self.shard = tiles_per_core * P
        self.n_pad = ncores * self.shard
        assert self.n_pad >= n_nodes
        self.bank_rows = bank_rows
        assert bank_rows % P == 0 and bank_rows <= 32768
        self.nbanks = (self.n_pad + bank_rows - 1) // bank_rows
        self.heads = list(heads)
        self.ch = list(ch)
        self.fin = [fin0, heads[0] * ch[0], heads[1] * ch[1]]
        # per-layer u16 table geometry: x cols (bf16) | al_src f32 | al_dst f32
        self.geom = []
        for l in range(3):
            xc = self.heads[l] * self.ch[l]
            elem = xc + 2 * self.heads[l]          # u16: x bf16 + al_src f32
            stride = ((xc + 4 * self.heads[l]) + 127) // 128 * 128
            self.geom.append(dict(xc=xc, elem=elem, stride=stride,
                                  alsrc=xc, aldst=xc + 2 * self.heads[l]))


# ----------------------------------------------------------------------------
# host-side graph preprocessing
# ----------------------------------------------------------------------------
def _wrap_idx(flat):
    """flat[e] (e = c*128 + p) -> [128, n/16] int16 tile for dma_gather.
    HW mapping (measured): out[p, c] = table[idx_sbuf[p % 16, p//16 + 8*c]]."""
    n = len(flat)
    B = flat.reshape(n // 128, 8, 16).transpose(2, 0, 1).reshape(16, n // 16)
    return np.tile(B, (8, 1))


def preprocess(cfg, edge_index):
    src = np.concatenate([np.asarray(edge_index[0]),
                          np.arange(cfg.n_nodes, dtype=np.int64)]).astype(np.int64)
    dst = np.concatenate([np.asarray(edge_index[1]),
                          np.arange(cfg.n_nodes, dtype=np.int64)]).astype(np.int64)
    ne = len(src)
    core = dst // cfg.shard
    tile_ic = (dst % cfg.shard) // P
    bank = src // cfg.bank_rows
    dst_local = (dst % P).astype(np.int32)
    src_local = (src - bank * cfg.bank_rows).astype(np.int32)

    ngroups = cfg.ncores * cfg.tiles * cfg.nbanks
    key = ((core * cfg.tiles + tile_ic) * cfg.nbanks + bank).astype(np.int64)
    cnt = np.bincount(key, minlength=ngroups).reshape(cfg.ncores, cfg.tiles, cfg.nbanks)
    ch_tb = (cnt.max(axis=0) + P - 1) // P          # [tiles, nbanks] chunks, shared
    assert ch_tb.max() <= 8, f"gather call would exceed 1024 idxs: {ch_tb.max()}"
    ni_tb = ch_tb * P
    stream_len = int(ni_tb.sum())                   # per-core padded edge stream

    # static offsets of each (t, b) group in the padded stream (t-major)
    off_tb = np.zeros((cfg.tiles, cfg.nbanks), np.int64)
    acc = 0
    for t in range(cfg.tiles):
        for b in range(cfg.nbanks):
            off_tb[t, b] = acc
            acc += ni_tb[t, b]

    # scatter edges into the padded per-core streams
    order = np.argsort(key, kind="stable")
    key_s = key[order]
    group_start = np.zeros(ngroups + 1, np.int64)
    np.cumsum(np.bincount(key_s, minlength=ngroups), out=group_start[1:])
    pos_in_group = np.arange(ne, dtype=np.int64) - group_start[key_s]
    tb_flat = key_s % (cfg.tiles * cfg.nbanks)
    t_of = tb_flat // cfg.nbanks
    b_of = tb_flat % cfg.nbanks
    stream_pos = off_tb[t_of, b_of] + pos_in_group
    core_s = key_s // (cfg.tiles * cfg.nbanks)

    srcl_pad = np.zeros((cfg.ncores, stream_len), np.int32)      # pad -> row 0
    dstl_pad = np.full((cfg.ncores, stream_len), 200, np.int32)  # pad sentinel
    srcl_pad[core_s, stream_pos] = src_local[order]
    dstl_pad[core_s, stream_pos] = dst_local[order]

    # per-tile chunk geometry
    nch_t = ch_tb.sum(axis=1).astype(np.int64)                  # chunks per tile
    nch_max = int(nch_t.max())
    meta = dict(ch_tb=ch_tb, ni_tb=ni_tb, off_tb=off_tb, nch_t=nch_t,
                nch_max=nch_max, stream_len=stream_len)

    # per-core flat device arrays; per-tile idx blocks are [128, nch_t*8]
    # (per-bank wrapped blocks concatenated on axis 1 -> one DMA per tile)
    idx_flats, dc_flats, dr_flats = [], [], []
    idx_off = np.zeros(cfg.tiles, np.int64)          # u16 offset of tile block
    idx_boff = np.zeros((cfg.tiles, cfg.nbanks), np.int64)   # col offset /16
    dc_off = np.zeros(cfg.tiles, np.int64)
    dr_off = np.zeros(cfg.tiles, np.int64)
    for c in range(cfg.ncores):
        idx_parts, dc_parts, dr_parts = [], [], []
        ioff = 0
        for t in range(cfg.tiles):
            seg0 = int(off_tb[t, 0])
            nt = int(nch_t[t]) * P
            seg = slice(seg0, seg0 + nt)
            dl = dstl_pad[c, seg]
            if c == 0:
                dc_off[t] = sum(x.size for x in dc_parts)
                dr_off[t] = sum(x.size for x in dr_parts)
                idx_off[t] = ioff
            dc_parts.append(dl.reshape(-1, P).T.astype(BF_NP).ravel())
            dr_parts.append(dl.astype(BF_NP))
            blocks = []
            coloff = 0
            for b in range(cfg.nbanks):
                ni = int(ni_tb[t, b])
                if ni == 0:
                    continue
                if c == 0:
                    idx_boff[t, b] = coloff
                blocks.append(_wrap_idx(
                    srcl_pad[c, int(off_tb[t, b]):int(off_tb[t, b]) + ni]
                    .astype(np.int16)))
                coloff += ni // 16
            tile_idx = np.concatenate(blocks, axis=1)   # [128, nch_t*8]
            idx_parts.append(tile_idx.ravel())
            ioff += tile_idx.size
        idx_flats.append(np.concatenate(idx_parts))
        dc_flats.append(np.concatenate(dc_parts))
        dr_flats.append(np.concatenate(dr_parts))
    meta.update(idx_off=idx_off, idx_boff=idx_boff, dc_off=dc_off, dr_off=dr_off,
                idx_len=len(idx_flats[0]), dc_len=len(dc_flats[0]),
                dr_len=len(dr_flats[0]))
    return meta, idx_flats, dc_flats, dr_flats


def make_weights(cfg, inputs):
    """Per-layer: W_x bf16 [fin, xc]; W_al f32 [fin, 2H]; b_rep f32 [128, xc]."""
    out = {}
    for l in range(3):
        W = np.asarray(inputs[f"W{l}"], np.float32)
        a_src = np.asarray(inputs[f"a_src{l}"], np.float32)
        a_dst = np.asarray(inputs[f"a_dst{l}"], np.float32)
        b = np.asarray(inputs[f"b{l}"], np.float32)
        H, C = a_src.shape
        wal = np.zeros((W.shape[0], 2 * H), np.float32)
        for h in range(H):
            wal[:, h] = W[:, h * C:(h + 1) * C] @ a_src[h]
            wal[:, H + h] = W[:, h * C:(h + 1) * C] @ a_dst[h]
        out[f"wx{l}"] = W.astype(BF_NP)
        out[f"wal{l}"] = wal
        out[f"brep{l}"] = np.broadcast_to(b, (P, len(b))).copy()
    return out


# ----------------------------------------------------------------------------
# patched dma_gather (non-transpose HBM source; elem bytes need not be %256)
# ----------------------------------------------------------------------------
def dma_gather_unaligned(gpsimd, out_ap, in_ap, idxs_ap, num_idxs, elem_size,
                         elem_step, queue_num=0):
    assert idxs_ap.dtype == I16
    assert in_ap.dtype == out_ap.dtype
    assert ap_utils.ap_is_contiguous(in_ap.ap[1:])
    assert ap_utils.ap_is_contiguous(out_ap.ap[1:])
    assert ap_utils.ap_is_contiguous(idxs_ap.ap[1:])
    assert in_ap.ap[-1][1] == out_ap.ap[-1][1] == elem_size
    assert out_ap.ap[0][1] * out_ap.ap[1][1] == (num_idxs + 127) // 128 * 128
    assert in_ap.ap[0][0] == elem_step
    dtsz = mybir.dt.size(in_ap.dtype)
    stride_bytes = elem_step * dtsz
    assert stride_bytes % 256 == 0 and stride_bytes // 256 < 256
    _in_ap = gpsimd.lower_ap_dma(in_ap, for_custom_bir_dma=True)
    _idxs_ap = gpsimd.lower_ap(idxs_ap)
    _out_ap = gpsimd.lower_ap(out_ap)
    return gpsimd.add_instruction(
        mybir.InstDMAGatherAnt(
            name=gpsimd.bass.get_next_instruction_name(),
            ins=[*_in_ap, _idxs_ap,
                 gpsimd.lower_val_access(gpsimd.to_reg(num_idxs))],
            outs=[_out_ap],
            transpose=False, num_idxs=num_idxs, elem_size=elem_size,
            stride_bytes_256=stride_bytes // 256, gen_mode=0,
            single_packet=True, queue_num=queue_num,
            sbuf_tokens_per_rank=0, sbuf_free_dim_per_rank=0,
            sbuf_free_dim_pad_per_rank=0, sbuf_byte_offset=0,
        ))


# ----------------------------------------------------------------------------
# kernel builder
# ----------------------------------------------------------------------------
def build(cfg, meta):
    nc = bacc.Bacc("TRN2", target_bir_lowering=False, debug=False,
                   num_devices=cfg.ncores, num_swdge_queues=4,
                   dynamic_dma_scratch_size=32768)
    g0 = cfg.geom[0]

    feats = nc.dram_tensor("feats", [cfg.shard, cfg.fin[0]], F32, kind="ExternalInput")
    idxs = nc.dram_tensor("idxs", [meta["idx_len"]], I16, kind="ExternalInput")
    dcol = nc.dram_tensor("dcol", [meta["dc_len"]], BF16, kind="ExternalInput")
    drow = nc.dram_tensor("drow", [meta["dr_len"]], BF16, kind="ExternalInput")
    wx, wal, brep = [], [], []
    for l in range(3):
        wx.append(nc.dram_tensor(f"wx{l}", [cfg.fin[l], cfg.geom[l]["xc"]], BF16,
                                 kind="ExternalInput"))
        wal.append(nc.dram_tensor(f"wal{l}", [cfg.fin[l], 2 * cfg.heads[l]], F32,
                                  kind="ExternalInput"))
        brep.append(nc.dram_tensor(f"brep{l}", [P, cfg.geom[l]["xc"]], F32,
                                   kind="ExternalInput"))
    ident_in = nc.dram_tensor("ident", [P, P], F32, kind="ExternalInput")
    iota_in = nc.dram_tensor("iota", [P, P], BF16, kind="ExternalInput")
    iotac_in = nc.dram_tensor("iotac", [P, 1], BF16, kind="ExternalInput")
    iotacf_in = nc.dram_tensor("iotacf", [P, 1], F32, kind="ExternalInput")
    ones_in = nc.dram_tensor("ones", [1, P], BF16, kind="ExternalInput")
    out_sh = nc.dram_tensor("out_shard", [cfg.shard, cfg.ch[2]], F32,
                            kind="ExternalOutput")

    NQ = int(os.environ.get("GAT_NQ", "4"))
    ch_tb, ni_tb, nch_t = meta["ch_tb"], meta["ni_tb"], meta["nch_t"]
    idx_off, idx_boff = meta["idx_off"], meta["idx_boff"]
    dc_off, dr_off = meta["dc_off"], meta["dr_off"]
    NCH = meta["nch_max"]

    with tile.TileContext(nc) as tc:
        with (
            tc.tile_pool(name="const", bufs=1) as cp,
            tc.tile_pool(name="sb", bufs=2) as sp,
            tc.tile_pool(name="ps", bufs=1, space="PSUM") as pp,
            tc.tile_pool(name="dram", bufs=1, space="DRAM") as dp,
        ):
            # ---------------- constants ----------------
            ident = cp.tile([P, P], F32)
            nc.sync.dma_start(out=ident[:], in_=ident_in[:, :])
            iota = cp.tile([P, P], BF16)
            nc.sync.dma_start(out=iota[:], in_=iota_in[:, :])
            iotac = cp.tile([P, 1], BF16)
            nc.sync.dma_start(out=iotac[:], in_=iotac_in[:, :])
            iotacf = cp.tile([P, 1], F32)
            nc.sync.dma_start(out=iotacf[:], in_=iotacf_in[:, :])
            ones = cp.tile([1, P], BF16)
            nc.sync.dma_start(out=ones[:], in_=ones_in[:, :])
            wx_t, wal_t, b_t = [], [], []
            for l in range(3):
                nf = cfg.fin[l] // P
                t = cp.tile([P, nf, cfg.geom[l]["xc"]], BF16, name=f"wxt{l}")
                nc.sync.dma_start(
                    out=t[:], in_=wx[l].ap().rearrange("(f p) c -> p f c", p=P))
                wx_t.append(t)
                t = cp.tile([P, nf, 2 * cfg.heads[l]], F32, name=f"walt{l}")
                nc.sync.dma_start(
                    out=t[:], in_=wal[l].ap().rearrange("(f p) c -> p f c", p=P))
                wal_t.append(t)
                t = cp.tile([P, cfg.geom[l]["xc"]], F32, name=f"bt{l}")
                nc.sync.dma_start(out=t[:], in_=brep[l].ap())
                b_t.append(t)

            # ---------------- DRAM bounces ----------------
            agin, table = [], []
            for l in range(3):
                st = cfg.geom[l]["stride"]
                agin.append(dp.tile([cfg.shard, st], U16, name=f"agin{l}"))
                table.append(dp.tile([cfg.n_pad, st], U16, name=f"table{l}",
                                     addr_space="Shared"))

            # ---------------- helpers ----------------
            def phase_a(l, t, h_tile):
                """h_tile: [128, fin] f32 SBUF -> writes agin[l] rows of tile t."""
                g = cfg.geom[l]
                nf = cfg.fin[l] // P
                hT = sp.tile([P, nf, P], F32, tag="hT")
                hTb = sp.tile([P, nf, P], BF16, tag="hTb")
                for f in range(nf):
                    tp = pp.tile([P, P], F32, space="PSUM", tag="scr", bufs=2)
                    nc.tensor.transpose(out=tp[:], in_=h_tile[:, f * P:(f + 1) * P],
                                        identity=ident[:])
                    nc.vector.tensor_copy(out=hT[:, f, :], in_=tp[:])
                    nc.scalar.activation(out=hTb[:, f, :], in_=hT[:, f, :],
                                         func=AF.Copy)
                aps = pp.tile([P, g["xc"] + 2 * cfg.heads[l]], F32, space="PSUM",
                              tag="aps")
                for f in range(nf):
                    nc.tensor.matmul(out=aps[:, 0:g["xc"]], lhsT=hTb[:, f, :],
                                     rhs=wx_t[l][:, f, :],
                                     start=(f == 0), stop=(f == nf - 1))
                for f in range(nf):
                    nc.tensor.matmul(out=aps[:, g["xc"]:], lhsT=hT[:, f, :],
                                     rhs=wal_t[l][:, f, :],
                                     start=(f == 0), stop=(f == nf - 1))
                row = sp.tile([P, g["stride"]], U16, tag="row")
                pad0 = g["xc"] + 4 * cfg.heads[l]
                if pad0 < g["stride"]:
                    nc.vector.memset(row[:, pad0:g["stride"]], 0)
                rb = row[:].bitcast(BF16)
                nc.scalar.activation(out=rb[:, 0:g["xc"]], in_=aps[:, 0:g["xc"]],
                                     func=AF.Copy)
                rf = row[:].bitcast(F32)
                H_ = cfg.heads[l]
                nc.vector.tensor_copy(out=rf[:, g["xc"] // 2:g["xc"] // 2 + H_],
                                      in_=aps[:, g["xc"]:g["xc"] + H_])
                # al_dst as bf16 hi/lo pair at u16 cols [xc+2H : xc+4H]
                hi_sl = rb[:, g["xc"] + 2 * H_:g["xc"] + 3 * H_]
                nc.scalar.activation(out=hi_sl, in_=aps[:, g["xc"] + H_:], func=AF.Copy)
                nc.vector.tensor_tensor(out=rb[:, g["xc"] + 3 * H_:g["xc"] + 4 * H_],
                                        in0=aps[:, g["xc"] + H_:], in1=hi_sl,
                                        op=OP.subtract)
                nc.scalar.dma_start(out=agin[l][t * P:(t + 1) * P, :], in_=row[:])

            def edge_loads(l, t):
                """stage 0: idx/dcol/drow/aldr loads + 4-bank gathers."""
                g = cfg.geom[l]
                H = cfg.heads[l]
                xc = g["xc"]
                nch = int(nch_t[t])
                E = nch * P

                gt = sp.tile([P, nch, g["elem"]], U16, tag="g", bufs=3)
                it = sp.tile([P, nch * 8], I16, tag="idx", bufs=6)
                nc.sync.dma_start(
                    out=it[:],
                    in_=idxs.ap()[int(idx_off[t]):int(idx_off[t]) + P * nch * 8]
                    .rearrange("(p m) -> p m", p=P))
                coff = 0
                for b in range(cfg.nbanks):
                    chb = int(ch_tb[t, b])
                    if chb == 0:
                        continue
                    ni = chb * P
                    rows = min(cfg.bank_rows, cfg.n_pad - b * cfg.bank_rows)
                    dma_gather_unaligned(
                        nc.gpsimd,
                        out_ap=gt[:, coff:coff + chb, :],
                        in_ap=table[l][b * cfg.bank_rows:b * cfg.bank_rows + rows,
                                       0:g["elem"]],
                        idxs_ap=it[:, int(idx_boff[t, b]):int(idx_boff[t, b]) + ni // 16],
                        num_idxs=ni, elem_size=g["elem"],
                        elem_step=g["stride"], queue_num=b % NQ)
                    coff += chb

                dcol_t = sp.tile([P, nch], BF16, tag="dcol", bufs=6)
                nc.sync.dma_start(
                    out=dcol_t[:],
                    in_=dcol.ap()[int(dc_off[t]):int(dc_off[t]) + P * nch]
                    .rearrange("(p m) -> p m", p=P))
                drow_t = sp.tile([1, E], BF16, tag="drow", bufs=3)
                nc.sync.dma_start(
                    out=drow_t[:],
                    in_=drow.ap()[int(dr_off[t]):int(dr_off[t]) + E].unsqueeze(0))
                hilo = sp.tile([P, 2 * H], BF16, tag="hilo", bufs=6)
                nc.sync.dma_start(
                    out=hilo[:].bitcast(U16),
                    in_=agin[l][t * P:(t + 1) * P, xc + 2 * H:xc + 4 * H])
                return dict(gt=gt, dcol_t=dcol_t, drow_t=drow_t, hilo=hilo)

            def edge_front(l, t, ld):
                """one-hot builds + al_dst expansion (deps: loads of t only)."""
                g = cfg.geom[l]
                H = cfg.heads[l]
                xc = g["xc"]
                nch = int(nch_t[t])
                E = nch * P
                dcol_t, drow_t, hilo = ld["dcol_t"], ld["drow_t"], ld["hilo"]

                oh = sp.tile([P, nch, P], BF16, tag="oh", bufs=3)
                nc.vector.tensor_tensor(
                    out=oh[:],
                    in0=dcol_t[:].unsqueeze(2).to_broadcast([P, nch, P]),
                    in1=iota[:].unsqueeze(1).to_broadcast([P, nch, P]),
                    op=OP.is_equal)
                dstb = sp.tile([P, E], BF16, tag="dstb", bufs=3)
                for pi, s0 in enumerate(range(0, E, 512)):
                    s1 = min(s0 + 512, E)
                    bc = pp.tile([P, 512], F32, space="PSUM", tag="scr", bufs=2)
                    nc.tensor.matmul(out=bc[:, 0:s1 - s0], lhsT=ones[:],
                                     rhs=drow_t[:, s0:s1], start=True, stop=True)
                    if pi % 2 == 0:
                        nc.scalar.activation(out=dstb[:, s0:s1],
                                             in_=bc[:, 0:s1 - s0], func=AF.Copy)
                    else:
                        nc.vector.tensor_copy(out=dstb[:, s0:s1],
                                              in_=bc[:, 0:s1 - s0])
                ohT = sp.tile([P, E], BF16, tag="ohT", bufs=3)
                nc.vector.tensor_scalar(out=ohT[:], in0=dstb[:],
                                        scalar1=iotacf[:, 0:1], scalar2=0.0,
                                        op0=OP.subtract, op1=OP.is_equal)
                adx = pp.tile([P, nch * H], F32, space="PSUM", tag="adx", bufs=2)
                for c in range(nch):
                    nc.tensor.matmul(out=adx[:, c * H:(c + 1) * H],
                                     lhsT=ohT[:, c * P:(c + 1) * P],
                                     rhs=hilo[:, 0:H], start=True, stop=False)
                    nc.tensor.matmul(out=adx[:, c * H:(c + 1) * H],
                                     lhsT=ohT[:, c * P:(c + 1) * P],
                                     rhs=hilo[:, H:2 * H], start=False, stop=True)
                return dict(oh=oh, adx=adx)

            def edge_back(l, t, ld, fr):
                """attention weights + weighted values + segment sums."""
                g = cfg.geom[l]
                H = cfg.heads[l]
                C = cfg.ch[l]
                xc = g["xc"]
                nch = int(nch_t[t])
                gt, oh, adx = ld["gt"], fr["oh"], fr["adx"]

                gf = gt[:].bitcast(F32)
                alsrc = gf[:, :, xc // 2:xc // 2 + H]
                S = sp.tile([P, nch, H], F32, tag="S", bufs=3)
                nc.vector.tensor_tensor(
                    out=S[:], in0=alsrc,
                    in1=adx[:].rearrange("p (c k) -> p c k", k=H),
                    op=OP.add)
                S2 = sp.tile([P, nch, H], F32, tag="S2", bufs=3)
                nc.vector.scalar_tensor_tensor(out=S2[:], in0=S[:], scalar=0.2,
                                               in1=S[:], op0=OP.mult, op1=OP.max)
                gb = gt[:].bitcast(BF16)
                v = sp.tile([P, nch, xc + H], BF16, tag="v", bufs=2)
                nc.scalar.activation(out=v[:, :, xc:xc + H], in_=S2[:],
                                     func=AF.Exp)
                wexp = sp.tile([P, nch, xc], BF16, tag="wexp", bufs=2)
                nc.scalar.activation(
                    out=wexp[:].rearrange("p c (h x) -> p c h x", h=H),
                    in_=S2[:].unsqueeze(3).to_broadcast([P, nch, H, C]),
                    func=AF.Exp)
                nc.vector.tensor_tensor(out=v[:, :, 0:xc], in0=gb[:, :, 0:xc],
                                        in1=wexp[:], op=OP.mult)
                ops = pp.tile([P, xc + H], F32, space="PSUM", tag="ops", bufs=2)
                for c in range(nch):
                    nc.tensor.matmul(out=ops[:], lhsT=oh[:, c, :],
                                     rhs=v[:, c, :],
                                     start=(c == 0), stop=(c == nch - 1))
                return ops

            def edge_epi(l, t, ops):
                """normalize + bias (+ ELU); returns h_next or writes out."""
                g = cfg.geom[l]
                H = cfg.heads[l]
                C = cfg.ch[l]
                xc = g["xc"]
                se = sp.tile([P, H], F32, tag="se", bufs=3)
                nc.vector.tensor_scalar_add(out=se[:], in0=ops[:, xc:xc + H],
                                            scalar1=1e-30)
                rs = sp.tile([P, H], F32, tag="rs", bufs=3)
                nc.vector.reciprocal(out=rs[:], in_=se[:])
                h1 = sp.tile([P, xc], F32, tag="h1", bufs=3)
                nc.vector.tensor_tensor(
                    out=h1[:].rearrange("p (h x) -> p h x", h=H),
                    in0=ops[:, 0:xc].rearrange("p (h x) -> p h x", h=H),
                    in1=rs[:].unsqueeze(2).to_broadcast([P, H, C]),
                    op=OP.mult)
                h2 = sp.tile([P, xc], F32, tag="h2", bufs=3)
                nc.vector.tensor_tensor(out=h2[:], in0=h1[:], in1=b_t[l][:],
                                        op=OP.add)
                if l == 2:
                    nc.scalar.dma_start(out=out_sh[t * P:(t + 1) * P, :], in_=h2[:])
                    return None
                m = sp.tile([P, xc], F32, tag="m", bufs=3)
                nc.vector.tensor_scalar_min(out=m[:], in0=h2[:], scalar1=0.0)
                nc.scalar.activation(out=m[:], in_=m[:], func=AF.Exp)
                hn = sp.tile([P, xc], F32, tag="hn", bufs=3)
                nc.vector.scalar_tensor_tensor(out=hn[:], in0=m[:], scalar=-1.0,
                                               in1=h2[:], op0=OP.add, op1=OP.max)
                return hn

            # ---------------- program ----------------
            rg = [list(range(cfg.ncores))]

            def edge_phase(l, next_l):
                lds, frs, opss, hns = {}, {}, {}, {}
                T = cfg.tiles
                for t in range(T + 3):
                    if t < T:
                        lds[t] = edge_loads(l, t)
                    if t - 1 >= 0 and t - 1 < T:
                        frs[t - 1] = edge_front(l, t - 1, lds[t - 1])
                    if t - 2 >= 0 and t - 2 < T:
                        u = t - 2
                        opss[u] = edge_back(l, u, lds.pop(u), frs.pop(u))
                    if t - 3 >= 0 and t - 3 < T:
                        u = t - 3
                        hn = edge_epi(l, u, opss.pop(u))
                        if next_l is not None:
                            phase_a(next_l, u, hn)

            # layer 0 phase A from features
            for t in range(cfg.tiles):
                h0 = sp.tile([P, cfg.fin[0]], F32, tag="h0")
                nc.sync.dma_start(out=h0[:], in_=feats.ap()[t * P:(t + 1) * P, :])
                phase_a(0, t, h0)
            nc.gpsimd.collective_compute(
                "AllGather", OP.bypass, ins=[agin[0][:].opt()],
                outs=[table[0][:].opt()], replica_groups=rg)
            edge_phase(0, 1)
            nc.gpsimd.collective_compute(
                "AllGather", OP.bypass, ins=[agin[1][:].opt()],
                outs=[table[1][:].opt()], replica_groups=rg)
            edge_phase(1, 2)
            nc.gpsimd.collective_compute(
                "AllGather", OP.bypass, ins=[agin[2][:].opt()],
                outs=[table[2][:].opt()], replica_groups=rg)
            edge_phase(2, None)

    nc.compile()
    return nc


# ----------------------------------------------------------------------------
# entry point
# ----------------------------------------------------------------------------
def run_gat(cfg, inputs, trace=False):
    meta, idx_flats, dc_flats, dr_flats = preprocess(cfg, inputs["edge_index"])
    wts = make_weights(cfg, inputs)
    feats = np.asarray(inputs["features"], np.float32)
    feats_pad = np.zeros((cfg.n_pad, cfg.fin[0]), np.float32)
    feats_pad[:cfg.n_nodes] = feats

    nc = build(cfg, meta)

    shared = dict(wts)
    shared["ident"] = np.eye(P, dtype=np.float32)
    shared["iota"] = np.broadcast_to(np.arange(P, dtype=np.float32), (P, P)).astype(BF_NP)
    shared["iotac"] = np.arange(P, dtype=np.float32).reshape(P, 1).astype(BF_NP)
    shared["iotacf"] = np.arange(P, dtype=np.float32).reshape(P, 1)
    shared["ones"] = np.ones((1, P), BF_NP)
    in_maps = []
    for c in range(cfg.ncores):
        m = dict(shared)
        m["feats"] = feats_pad[c * cfg.shard:(c + 1) * cfg.shard]
        m["idxs"] = idx_flats[c]
        m["dcol"] = dc_flats[c]
        m["drow"] = dr_flats[c]
        in_maps.append(m)

    res = run_bass_kernel_spmd(nc, in_maps, core_ids=list(range(cfg.ncores)),
                               trace=trace)
    LAST_RESULT["exec_time_ns"] = res.exec_time_ns
    out = np.concatenate([res.results[c]["out_shard"] for c in range(cfg.ncores)],
                         axis=0)[:cfg.n_nodes]
    return out


def kernel(**inputs):
    cfg = Cfg()
    trace = os.environ.get("GAT_TRACE", "0") == "1"
    if trace:
        try:
            import sys as _sys, types as _types
            import trn_agent_boot.trn_boot as _tb
            _m = _types.ModuleType("antenv.axon_hooks")
            _hook = _tb._ntff_profile_via_ctypes("/opt/axon/libaxon_pjrt.so")
            _m.get_axon_ntff_profile_hook = lambda: _hook
            _m.set_axon_ntff_profile_hook = lambda h: None
            _sys.modules.setdefault("antenv.axon_hooks", _m)
            import concourse.bass_utils as _bu
            _bu.upload_artifacts = lambda tmpdir: f"file://{tmpdir}"
        except Exception:
            trace = False
    return run_gat(cfg, inputs, trace=trace).astype(np.float32)



# revision 4
# speedup vs baseline: 1.2777x; 1.2777x over previous
"""3-layer GAT (PyG GATConv semantics) forward on 8 Trainium2 NeuronCores.

Strategy (graph/data parallel, dst-sharded):
  - Nodes padded to N_PAD = 8*98*128 and sharded by destination across 8 cores.
  - Edges (plus self-loops) bucketed host-side by (core, dst-tile, src-bank),
    sorted by dst, padded to 128-edge chunks; chunk structure equalized across
    cores so one SPMD program serves all 8.
  - Per layer: each core computes x_aug = h @ [W | W*a_src | W*a_dst] for its
    node shard (x in bf16, attention logits in f32), AllGathers the packed
    row table, then processes its dst tiles: dma_gather (4 SWDGE queues)
    fetches x_aug rows by src, attention weights are computed with the
    exp(leaky_relu(al_src + al_dst)) folded per edge, and the per-dst softmax
    numerator/denominator are accumulated with one-hot matmuls on TensorE.
    al_dst is expanded dst->edges with a transposed one-hot matmul (hi/lo bf16
    split keeps f32 precision).
"""
import os
import numpy as np
import ml_dtypes

import concourse.bass as bass
import concourse.bacc as bacc
import concourse.tile as tile
import concourse.mybir as mybir
from concourse import ap_utils
from concourse.bass_utils import run_bass_kernel_spmd

F32 = mybir.dt.float32
BF16 = mybir.dt.bfloat16
U16 = mybir.dt.uint16
I16 = mybir.dt.int16
AF = mybir.ActivationFunctionType
OP = mybir.AluOpType
P = 128
BF_NP = ml_dtypes.bfloat16

LAST_RESULT = {}


# ----------------------------------------------------------------------------
# configuration
# ----------------------------------------------------------------------------
class Cfg:
    def __init__(self, n_nodes=100000, tiles_per_core=98, bank_rows=25088,
                 ncores=8, heads=(8, 8, 1), ch=(32, 32, 40), fin0=128):
        self.n_nodes = n_nodes
        self.ncores = ncores
        self.tiles = tiles_per_core
        self.shard = tiles_per_core * P
        self.n_pad = ncores * self.shard
        assert self.n_pad >= n_nodes
        self.bank_rows = bank_rows
        assert bank_rows % P == 0 and bank_rows <= 32768
        self.nbanks = (self.n_pad + bank_rows - 1) // bank_rows
        self.heads = list(heads)
        self.ch = list(ch)
        self.fin = [fin0, heads[0] * ch[0], heads[1] * ch[1]]
        # per-layer u16 table geometry: x cols (bf16) | al_src f32 | al_dst f32
        self.geom = []
        for l in range(3):
            xc = self.heads[l] * self.ch[l]
            elem = xc + 2 * self.heads[l]          # u16: x bf16 + al_src f32
            stride = ((xc + 4 * self.heads[l]) + 127) // 128 * 128
            self.geom.append(dict(xc=xc, elem=elem, stride=stride,
                                  alsrc=xc, aldst=xc + 2 * self.heads[l]))


# ----------------------------------------------------------------------------
# host-side graph preprocessing
# ----------------------------------------------------------------------------
def _wrap_idx(flat):
    """flat[e] (e = c*128 + p) -> [128, n/16] int16 tile for dma_gather.
    HW mapping (measured): out[p, c] = table[idx_sbuf[p % 16, p//16 + 8*c]]."""
    n = len(flat)
    B = flat.reshape(n // 128, 8, 16).transpose(2, 0, 1).reshape(16, n // 16)
    return np.tile(B, (8, 1))


def preprocess(cfg, edge_index):
    src = np.concatenate([np.asarray(edge_index[0]),
                          np.arange(cfg.n_nodes, dtype=np.int64)]).astype(np.int64)
    dst = np.concatenate([np.asarray(edge_index[1]),
                          np.arange(cfg.n_nodes, dtype=np.int64)]).astype(np.int64)
    ne = len(src)
    core = dst // cfg.shard
    tile_ic = (dst % cfg.shard) // P
    bank = src // cfg.bank_rows
    dst_local = (dst % P).astype(np.int32)
    src_local = (src - bank * cfg.bank_rows).astype(np.int32)

    ngroups = cfg.ncores * cfg.tiles * cfg.nbanks
    key = ((core * cfg.tiles + tile_ic) * cfg.nbanks + bank).astype(np.int64)
    cnt = np.bincount(key, minlength=ngroups).reshape(cfg.ncores, cfg.tiles, cfg.nbanks)
    ch_tb = (cnt.max(axis=0) + P - 1) // P          # [tiles, nbanks] chunks, shared
    assert ch_tb.max() <= 8, f"gather call would exceed 1024 idxs: {ch_tb.max()}"
    ni_tb = ch_tb * P
    stream_len = int(ni_tb.sum())                   # per-core padded edge stream

    # static offsets of each (t, b) group in the padded stream (t-major)
    off_tb = np.zeros((cfg.tiles, cfg.nbanks), np.int64)
    acc = 0
    for t in range(cfg.tiles):
        for b in range(cfg.nbanks):
            off_tb[t, b] = acc
            acc += ni_tb[t, b]

    # scatter edges into the padded per-core streams
    order = np.argsort(key, kind="stable")
    key_s = key[order]
    group_start = np.zeros(ngroups + 1, np.int64)
    np.cumsum(np.bincount(key_s, minlength=ngroups), out=group_start[1:])
    pos_in_group = np.arange(ne, dtype=np.int64) - group_start[key_s]
    tb_flat = key_s % (cfg.tiles * cfg.nbanks)
    t_of = tb_flat // cfg.nbanks
    b_of = tb_flat % cfg.nbanks
    stream_pos = off_tb[t_of, b_of] + pos_in_group
    core_s = key_s // (cfg.tiles * cfg.nbanks)

    srcl_pad = np.zeros((cfg.ncores, stream_len), np.int32)      # pad -> row 0
    dstl_pad = np.full((cfg.ncores, stream_len), 200, np.int32)  # pad sentinel
    srcl_pad[core_s, stream_pos] = src_local[order]
    dstl_pad[core_s, stream_pos] = dst_local[order]

    # per-tile chunk geometry
    nch_t = ch_tb.sum(axis=1).astype(np.int64)                  # chunks per tile
    nch_max = int(nch_t.max())
    meta = dict(ch_tb=ch_tb, ni_tb=ni_tb, off_tb=off_tb, nch_t=nch_t,
                nch_max=nch_max, stream_len=stream_len)

    # per-core flat device arrays; per-tile idx blocks are [128, nch_t*8]
    # (per-bank wrapped blocks concatenated on axis 1 -> one DMA per tile)
    idx_flats, dc_flats, dr_flats = [], [], []
    idx_off = np.zeros(cfg.tiles, np.int64)          # u16 offset of tile block
    idx_boff = np.zeros((cfg.tiles, cfg.nbanks), np.int64)   # col offset /16
    dc_off = np.zeros(cfg.tiles, np.int64)
    dr_off = np.zeros(cfg.tiles, np.int64)
    for c in range(cfg.ncores):
        idx_parts, dc_parts, dr_parts = [], [], []
        ioff = 0
        for t in range(cfg.tiles):
            seg0 = int(off_tb[t, 0])
            nt = int(nch_t[t]) * P
            seg = slice(seg0, seg0 + nt)
            dl = dstl_pad[c, seg]
            if c == 0:
                dc_off[t] = sum(x.size for x in dc_parts)
                dr_off[t] = sum(x.size for x in dr_parts)
                idx_off[t] = ioff
            dc_parts.append(dl.reshape(-1, P).T.astype(BF_NP).ravel())
            dr_parts.append(dl.astype(BF_NP))
            blocks = []
            coloff = 0
            for b in range(cfg.nbanks):
                ni = int(ni_tb[t, b])
                if ni == 0:
                    continue
                if c == 0:
                    idx_boff[t, b] = coloff
                blocks.append(_wrap_idx(
                    srcl_pad[c, int(off_tb[t, b]):int(off_tb[t, b]) + ni]
                    .astype(np.int16)))
                coloff += ni // 16
            tile_idx = np.concatenate(blocks, axis=1)   # [128, nch_t*8]
            idx_parts.append(tile_idx.ravel())
            ioff += tile_idx.size
        idx_flats.append(np.concatenate(idx_parts))
        dc_flats.append(np.concatenate(dc_parts))
        dr_flats.append(np.concatenate(dr_parts))
    meta.update(idx_off=idx_off, idx_boff=idx_boff, dc_off=dc_off, dr_off=dr_off,
                idx_len=len(idx_flats[0]), dc_len=len(dc_flats[0]),
                dr_len=len(dr_flats[0]))
    return meta, idx_flats, dc_flats, dr_flats


def make_weights(cfg, inputs):
    """Per-layer: W_x bf16 [fin, xc]; W_al f32 [fin, 2H]; b_rep f32 [128, xc]."""
    out = {}
    for l in range(3):
        W = np.asarray(inputs[f"W{l}"], np.float32)
        a_src = np.asarray(inputs[f"a_src{l}"], np.float32)
        a_dst = np.asarray(inputs[f"a_dst{l}"], np.float32)
        b = np.asarray(inputs[f"b{l}"], np.float32)
        H, C = a_src.shape
        wal = np.zeros((W.shape[0], 2 * H), np.float32)
        for h in range(H):
            wal[:, h] = W[:, h * C:(h + 1) * C] @ a_src[h]
            wal[:, H + h] = W[:, h * C:(h + 1) * C] @ a_dst[h]
        out[f"wx{l}"] = W.astype(BF_NP)
        out[f"wal{l}"] = wal
        out[f"brep{l}"] = np.broadcast_to(b, (P, len(b))).copy()
    return out


# ----------------------------------------------------------------------------
# patched dma_gather (non-transpose HBM source; elem bytes need not be %256)
# ----------------------------------------------------------------------------
def dma_gather_unaligned(gpsimd, out_ap, in_ap, idxs_ap, num_idxs, elem_size,
                         elem_step, queue_num=0):
    assert idxs_ap.dtype == I16
    assert in_ap.dtype == out_ap.dtype
    assert ap_utils.ap_is_contiguous(in_ap.ap[1:])
    assert ap_utils.ap_is_contiguous(out_ap.ap[1:])
    assert ap_utils.ap_is_contiguous(idxs_ap.ap[1:])
    assert in_ap.ap[-1][1] == out_ap.ap[-1][1] == elem_size
    assert out_ap.ap[0][1] * out_ap.ap[1][1] == (num_idxs + 127) // 128 * 128
    assert in_ap.ap[0][0] == elem_step
    dtsz = mybir.dt.size(in_ap.dtype)
    stride_bytes = elem_step * dtsz
    assert stride_bytes % 256 == 0 and stride_bytes // 256 < 256
    _in_ap = gpsimd.lower_ap_dma(in_ap, for_custom_bir_dma=True)
    _idxs_ap = gpsimd.lower_ap(idxs_ap)
    _out_ap = gpsimd.lower_ap(out_ap)
    return gpsimd.add_instruction(
        mybir.InstDMAGatherAnt(
            name=gpsimd.bass.get_next_instruction_name(),
            ins=[*_in_ap, _idxs_ap,
                 gpsimd.lower_val_access(gpsimd.to_reg(num_idxs))],
            outs=[_out_ap],
            transpose=False, num_idxs=num_idxs, elem_size=elem_size,
            stride_bytes_256=stride_bytes // 256, gen_mode=0,
            single_packet=True, queue_num=queue_num,
            sbuf_tokens_per_rank=0, sbuf_free_dim_per_rank=0,
            sbuf_free_dim_pad_per_rank=0, sbuf_byte_offset=0,
        ))


# ----------------------------------------------------------------------------
# kernel builder
# ----------------------------------------------------------------------------
def build(cfg, meta):
    nc = bacc.Bacc("TRN2", target_bir_lowering=False, debug=False,
                   num_devices=cfg.ncores, num_swdge_queues=4,
                   dynamic_dma_scratch_size=32768)
    g0 = cfg.geom[0]

    feats = nc.dram_tensor("feats", [cfg.shard, cfg.fin[0]], F32, kind="ExternalInput")
    idxs = nc.dram_tensor("idxs", [meta["idx_len"]], I16, kind="ExternalInput")
    dcol = nc.dram_tensor("dcol", [meta["dc_len"]], BF16, kind="ExternalInput")
    drow = nc.dram_tensor("drow", [meta["dr_len"]], BF16, kind="ExternalInput")
    wx, wal, brep = [], [], []
    for l in range(3):
        wx.append(nc.dram_tensor(f"wx{l}", [cfg.fin[l], cfg.geom[l]["xc"]], BF16,
                                 kind="ExternalInput"))
        wal.append(nc.dram_tensor(f"wal{l}", [cfg.fin[l], 2 * cfg.heads[l]], F32,
                                  kind="ExternalInput"))
        brep.append(nc.dram_tensor(f"brep{l}", [P, cfg.geom[l]["xc"]], F32,
                                   kind="ExternalInput"))
    ident_in = nc.dram_tensor("ident", [P, P], F32, kind="ExternalInput")
    iota_in = nc.dram_tensor("iota", [P, P], BF16, kind="ExternalInput")
    iotac_in = nc.dram_tensor("iotac", [P, 1], BF16, kind="ExternalInput")
    iotacf_in = nc.dram_tensor("iotacf", [P, 1], F32, kind="ExternalInput")
    ones_in = nc.dram_tensor("ones", [1, P], BF16, kind="ExternalInput")
    out_sh = nc.dram_tensor("out_shard", [cfg.shard, cfg.ch[2]], F32,
                            kind="ExternalOutput")

    NQ = int(os.environ.get("GAT_NQ", "4"))
    ch_tb, ni_tb, nch_t = meta["ch_tb"], meta["ni_tb"], meta["nch_t"]
    idx_off, idx_boff = meta["idx_off"], meta["idx_boff"]
    dc_off, dr_off = meta["dc_off"], meta["dr_off"]
    NCH = meta["nch_max"]

    with tile.TileContext(nc) as tc:
        with (
            tc.tile_pool(name="const", bufs=1) as cp,
            tc.tile_pool(name="sb", bufs=2) as sp,
            tc.tile_pool(name="ps", bufs=1, space="PSUM") as pp,
            tc.tile_pool(name="dram", bufs=1, space="DRAM") as dp,
        ):
            # ---------------- constants ----------------
            ident = cp.tile([P, P], F32)
            nc.sync.dma_start(out=ident[:], in_=ident_in[:, :])
            iota = cp.tile([P, P], BF16)
            nc.sync.dma_start(out=iota[:], in_=iota_in[:, :])
            iotac = cp.tile([P, 1], BF16)
            nc.sync.dma_start(out=iotac[:], in_=iotac_in[:, :])
            iotacf = cp.tile([P, 1], F32)
            nc.sync.dma_start(out=iotacf[:], in_=iotacf_in[:, :])
            ones = cp.tile([1, P], BF16)
            nc.sync.dma_start(out=ones[:], in_=ones_in[:, :])
            wx_t, wal_t, b_t = [], [], []
            for l in range(3):
                nf = cfg.fin[l] // P
                t = cp.tile([P, nf, cfg.geom[l]["xc"]], BF16, name=f"wxt{l}")
                nc.sync.dma_start(
                    out=t[:], in_=wx[l].ap().rearrange("(f p) c -> p f c", p=P))
                wx_t.append(t)
                t = cp.tile([P, nf, 2 * cfg.heads[l]], F32, name=f"walt{l}")
                nc.sync.dma_start(
                    out=t[:], in_=wal[l].ap().rearrange("(f p) c -> p f c", p=P))
                wal_t.append(t)
                t = cp.tile([P, cfg.geom[l]["xc"]], F32, name=f"bt{l}")
                nc.sync.dma_start(out=t[:], in_=brep[l].ap())
                b_t.append(t)

            # ---------------- DRAM bounces ----------------
            agin, table = [], []
            for l in range(3):
                st = cfg.geom[l]["stride"]
                agin.append(dp.tile([cfg.shard, st], U16, name=f"agin{l}"))
                table.append(dp.tile([cfg.n_pad, st], U16, name=f"table{l}",
                                     addr_space="Shared"))

            # ---------------- helpers ----------------
            def phase_a(l, t, h_tile):
                """h_tile: [128, fin] f32 SBUF -> writes agin[l] rows of tile t."""
                g = cfg.geom[l]
                nf = cfg.fin[l] // P
                hT = sp.tile([P, nf, P], F32, tag="hT")
                hTb = sp.tile([P, nf, P], BF16, tag="hTb")
                for f in range(nf):
                    tp = pp.tile([P, P], F32, space="PSUM", tag="scr", bufs=2)
                    nc.tensor.transpose(out=tp[:], in_=h_tile[:, f * P:(f + 1) * P],
                                        identity=ident[:])
                    nc.vector.tensor_copy(out=hT[:, f, :], in_=tp[:])
                    nc.scalar.activation(out=hTb[:, f, :], in_=hT[:, f, :],
                                         func=AF.Copy)
                aps = pp.tile([P, g["xc"] + 2 * cfg.heads[l]], F32, space="PSUM",
                              tag="aps")
                for f in range(nf):
                    nc.tensor.matmul(out=aps[:, 0:g["xc"]], lhsT=hTb[:, f, :],
                                     rhs=wx_t[l][:, f, :],
                                     start=(f == 0), stop=(f == nf - 1))
                for f in range(nf):
                    nc.tensor.matmul(out=aps[:, g["xc"]:], lhsT=hT[:, f, :],
                                     rhs=wal_t[l][:, f, :],
                                     start=(f == 0), stop=(f == nf - 1))
                row = sp.tile([P, g["stride"]], U16, tag="row")
                pad0 = g["xc"] + 4 * cfg.heads[l]
                if pad0 < g["stride"]:
                    nc.vector.memset(row[:, pad0:g["stride"]], 0)
                rb = row[:].bitcast(BF16)
                nc.scalar.activation(out=rb[:, 0:g["xc"]], in_=aps[:, 0:g["xc"]],
                                     func=AF.Copy)
                rf = row[:].bitcast(F32)
                H_ = cfg.heads[l]
                nc.vector.tensor_copy(out=rf[:, g["xc"] // 2:g["xc"] // 2 + H_],
                                      in_=aps[:, g["xc"]:g["xc"] + H_])
                # al_dst as bf16 hi/lo pair at u16 cols [xc+2H : xc+4H]
                hi_sl = rb[:, g["xc"] + 2 * H_:g["xc"] + 3 * H_]
                nc.scalar.activation(out=hi_sl, in_=aps[:, g["xc"] + H_:], func=AF.Copy)
                nc.vector.tensor_tensor(out=rb[:, g["xc"] + 3 * H_:g["xc"] + 4 * H_],
                                        in0=aps[:, g["xc"] + H_:], in1=hi_sl,
                                        op=OP.subtract)
                nc.scalar.dma_start(out=agin[l][t * P:(t + 1) * P, :], in_=row[:])

            def edge_loads(l, t):
                """stage 0: idx/dcol/drow/aldr loads + 4-bank gathers."""
                g = cfg.geom[l]
                H = cfg.heads[l]
                xc = g["xc"]
                nch = int(nch_t[t])
                E = nch * P

                gt = sp.tile([P, nch, g["elem"]], U16, tag="g", bufs=3)
                it = sp.tile([P, nch * 8], I16, tag="idx", bufs=6)
                nc.sync.dma_start(
                    out=it[:],
                    in_=idxs.ap()[int(idx_off[t]):int(idx_off[t]) + P * nch * 8]
                    .rearrange("(p m) -> p m", p=P))
                coff = 0
                for b in range(cfg.nbanks):
                    chb = int(ch_tb[t, b])
                    if chb == 0:
                        continue
                    ni = chb * P
                    rows = min(cfg.bank_rows, cfg.n_pad - b * cfg.bank_rows)
                    dma_gather_unaligned(
                        nc.gpsimd,
                        out_ap=gt[:, coff:coff + chb, :],
                        in_ap=table[l][b * cfg.bank_rows:b * cfg.bank_rows + rows,
                                       0:g["elem"]],
                        idxs_ap=it[:, int(idx_boff[t, b]):int(idx_boff[t, b]) + ni // 16],
                        num_idxs=ni, elem_size=g["elem"],
                        elem_step=g["stride"], queue_num=b % NQ)
                    coff += chb

                dcol_t = sp.tile([P, nch], BF16, tag="dcol", bufs=6)
                nc.sync.dma_start(
                    out=dcol_t[:],
                    in_=dcol.ap()[int(dc_off[t]):int(dc_off[t]) + P * nch]
                    .rearrange("(p m) -> p m", p=P))
                drow_t = sp.tile([1, E], BF16, tag="drow", bufs=3)
                nc.sync.dma_start(
                    out=drow_t[:],
                    in_=drow.ap()[int(dr_off[t]):int(dr_off[t]) + E].unsqueeze(0))
                hilo = sp.tile([P, 2 * H], BF16, tag="hilo", bufs=6)
                nc.sync.dma_start(
                    out=hilo[:].bitcast(U16),
                    in_=agin[l][t * P:(t + 1) * P, xc + 2 * H:xc + 4 * H])
                return dict(gt=gt, dcol_t=dcol_t, drow_t=drow_t, hilo=hilo)

            def edge_front(l, t, ld):
                """one-hot builds + al_dst expansion (deps: loads of t only)."""
                g = cfg.geom[l]
                H = cfg.heads[l]
                xc = g["xc"]
                nch = int(nch_t[t])
                E = nch * P
                dcol_t, drow_t, hilo = ld["dcol_t"], ld["drow_t"], ld["hilo"]

                oh = sp.tile([P, nch, P], BF16, tag="oh", bufs=3)
                nc.vector.tensor_tensor(
                    out=oh[:],
                    in0=dcol_t[:].unsqueeze(2).to_broadcast([P, nch, P]),
                    in1=iota[:].unsqueeze(1).to_broadcast([P, nch, P]),
                    op=OP.is_equal)
                dstb = sp.tile([P, E], BF16, tag="dstb", bufs=3)
                for pi, s0 in enumerate(range(0, E, 512)):
                    s1 = min(s0 + 512, E)
                    bc = pp.tile([P, 512], F32, space="PSUM", tag="scr", bufs=2)
                    nc.tensor.matmul(out=bc[:, 0:s1 - s0], lhsT=ones[:],
                                     rhs=drow_t[:, s0:s1], start=True, stop=True)
                    if pi % 2 == 0:
                        nc.scalar.activation(out=dstb[:, s0:s1],
                                             in_=bc[:, 0:s1 - s0], func=AF.Copy)
                    else:
                        nc.vector.tensor_copy(out=dstb[:, s0:s1],
                                              in_=bc[:, 0:s1 - s0])
                ohT = sp.tile([P, E], BF16, tag="ohT", bufs=3)
                nc.vector.tensor_scalar(out=ohT[:], in0=dstb[:],
                                        scalar1=iotacf[:, 0:1], scalar2=0.0,
                                        op0=OP.subtract, op1=OP.is_equal)
                adx = pp.tile([P, nch * 2 * H], F32, space="PSUM", tag="adx", bufs=2)
                for c in range(nch):
                    nc.tensor.matmul(out=adx[:, c * 2 * H:(c + 1) * 2 * H],
                                     lhsT=ohT[:, c * P:(c + 1) * P],
                                     rhs=hilo[:], start=True, stop=True)
                return dict(oh=oh, adx=adx)

            def edge_back(l, t, ld, fr):
                """attention weights + weighted values + segment sums."""
                g = cfg.geom[l]
                H = cfg.heads[l]
                C = cfg.ch[l]
                xc = g["xc"]
                nch = int(nch_t[t])
                gt, oh, adx = ld["gt"], fr["oh"], fr["adx"]

                gf = gt[:].bitcast(F32)
                alsrc = gf[:, :, xc // 2:xc // 2 + H]
                a2 = adx[:].rearrange("p (c k) -> p c k", k=2 * H)
                S = sp.tile([P, nch, H], F32, tag="S", bufs=3)
                nc.vector.tensor_tensor(out=S[:], in0=alsrc, in1=a2[:, :, 0:H],
                                        op=OP.add)
                nc.vector.tensor_tensor(out=S[:], in0=S[:], in1=a2[:, :, H:2 * H],
                                        op=OP.add)
                S2 = sp.tile([P, nch, H], F32, tag="S2", bufs=3)
                nc.vector.scalar_tensor_tensor(out=S2[:], in0=S[:], scalar=0.2,
                                               in1=S[:], op0=OP.mult, op1=OP.max)
                gb = gt[:].bitcast(BF16)
                v = sp.tile([P, nch, xc + H], BF16, tag="v", bufs=2)
                nc.scalar.activation(out=v[:, :, xc:xc + H], in_=S2[:],
                                     func=AF.Exp)
                nc.vector.tensor_tensor(
                    out=v[:, :, 0:xc].rearrange("p c (h x) -> p c h x", h=H),
                    in0=gb[:, :, 0:xc].rearrange("p c (h x) -> p c h x", h=H),
                    in1=v[:, :, xc:xc + H].unsqueeze(3).to_broadcast([P, nch, H, C]),
                    op=OP.mult)
                ops = pp.tile([P, xc + H], F32, space="PSUM", tag="ops", bufs=2)
                for c in range(nch):
                    nc.tensor.matmul(out=ops[:], lhsT=oh[:, c, :],
                                     rhs=v[:, c, :],
                                     start=(c == 0), stop=(c == nch - 1))
                return ops

            def edge_epi(l, t, ops):
                """normalize + bias (+ ELU); returns h_next or writes out."""
                g = cfg.geom[l]
                H = cfg.heads[l]
                C = cfg.ch[l]
                xc = g["xc"]
                se = sp.tile([P, H], F32, tag="se", bufs=3)
                nc.vector.tensor_scalar_add(out=se[:], in0=ops[:, xc:xc + H],
                                            scalar1=1e-30)
                rs = sp.tile([P, H], F32, tag="rs", bufs=3)
                nc.vector.reciprocal(out=rs[:], in_=se[:])
                h1 = sp.tile([P, xc], F32, tag="h1", bufs=3)
                nc.vector.tensor_tensor(
                    out=h1[:].rearrange("p (h x) -> p h x", h=H),
                    in0=ops[:, 0:xc].rearrange("p (h x) -> p h x", h=H),
                    in1=rs[:].unsqueeze(2).to_broadcast([P, H, C]),
                    op=OP.mult)
                h2 = sp.tile([P, xc], F32, tag="h2", bufs=3)
                nc.vector.tensor_tensor(out=h2[:], in0=h1[:], in1=b_t[l][:],
                                        op=OP.add)
                if l == 2:
                    nc.scalar.dma_start(out=out_sh[t * P:(t + 1) * P, :], in_=h2[:])
                    return None
                m = sp.tile([P, xc], F32, tag="m", bufs=3)
                nc.vector.tensor_scalar_min(out=m[:], in0=h2[:], scalar1=0.0)
                nc.scalar.activation(out=m[:], in_=m[:], func=AF.Exp)
                hn = sp.tile([P, xc], F32, tag="hn", bufs=3)
                nc.vector.scalar_tensor_tensor(out=hn[:], in0=m[:], scalar=-1.0,
                                               in1=h2[:], op0=OP.add, op1=OP.max)
                return hn

            # ---------------- program ----------------
            rg = [list(range(cfg.ncores))]

            def edge_phase(l, next_l):
                lds, frs, opss, hns = {}, {}, {}, {}
                T = cfg.tiles
                for t in range(T + 3):
                    if t < T:
                        lds[t] = edge_loads(l, t)
                    if t - 1 >= 0 and t - 1 < T:
                        frs[t - 1] = edge_front(l, t - 1, lds[t - 1])
                    if t - 2 >= 0 and t - 2 < T:
                        u = t - 2
                        opss[u] = edge_back(l, u, lds.pop(u), frs.pop(u))
                    if t - 3 >= 0 and t - 3 < T:
                        u = t - 3
                        hn = edge_epi(l, u, opss.pop(u))
                        if next_l is not None:
                            phase_a(next_l, u, hn)

            # layer 0 phase A from features
            for t in range(cfg.tiles):
                h0 = sp.tile([P, cfg.fin[0]], F32, tag="h0")
                nc.sync.dma_start(out=h0[:], in_=feats.ap()[t * P:(t + 1) * P, :])
                phase_a(0, t, h0)
            nc.gpsimd.collective_compute(
                "AllGather", OP.bypass, ins=[agin[0][:].opt()],
                outs=[table[0][:].opt()], replica_groups=rg)
            edge_phase(0, 1)
            nc.gpsimd.collective_compute(
                "AllGather", OP.bypass, ins=[agin[1][:].opt()],
                outs=[table[1][:].opt()], replica_groups=rg)
            edge_phase(1, 2)
            nc.gpsimd.collective_compute(
                "AllGather", OP.bypass, ins=[agin[2][:].opt()],
                outs=[table[2][:].opt()], replica_groups=rg)
            edge_phase(2, None)

    nc.compile()
    return nc


# ----------------------------------------------------------------------------
# entry point
# ----------------------------------------------------------------------------
def run_gat(cfg, inputs, trace=False):
    meta, idx_flats, dc_flats, dr_flats = preprocess(cfg, inputs["edge_index"])
    wts = make_weights(cfg, inputs)
    feats = np.asarray(inputs["features"], np.float32)
    feats_pad = np.zeros((cfg.n_pad, cfg.fin[0]), np.float32)
    feats_pad[:cfg.n_nodes] = feats

    nc = build(cfg, meta)

    shared = dict(wts)
    shared["ident"] = np.eye(P, dtype=np.float32)
    shared["iota"] = np.broadcast_to(np.arange(P, dtype=np.float32), (P, P)).astype(BF_NP)
    shared["iotac"] = np.arange(P, dtype=np.float32).reshape(P, 1).astype(BF_NP)
    shared["iotacf"] = np.arange(P, dtype=np.float32).reshape(P, 1)
    shared["ones"] = np.ones((1, P), BF_NP)
    in_maps = []
    for c in range(cfg.ncores):
        m = dict(shared)
        m["feats"] = feats_pad[c * cfg.shard:(c + 1) * cfg.shard]
        m["idxs"] = idx_flats[c]
        m["dcol"] = dc_flats[c]
        m["drow"] = dr_flats[c]
        in_maps.append(m)

    res = run_bass_kernel_spmd(nc, in_maps, core_ids=list(range(cfg.ncores)),
                               trace=trace)
    LAST_RESULT["exec_time_ns"] = res.exec_time_ns
    out = np.concatenate([res.results[c]["out_shard"] for c in range(cfg.ncores)],
                         axis=0)[:cfg.n_nodes]
    return out


def kernel(**inputs):
    cfg = Cfg()
    trace = os.environ.get("GAT_TRACE", "0") == "1"
    if trace:
        try:
            import sys as _sys, types as _types
            import trn_agent_boot.trn_boot as _tb
            _m = _types.ModuleType("antenv.axon_hooks")
            _hook = _tb._ntff_profile_via_ctypes("/opt/axon/libaxon_pjrt.so")
            _m.get_axon_ntff_profile_hook = lambda: _hook
            _m.set_axon_ntff_profile_hook = lambda h: None
            _sys.modules.setdefault("antenv.axon_hooks", _m)
            import concourse.bass_utils as _bu
            _bu.upload_artifacts = lambda tmpdir: f"file://{tmpdir}"
        except Exception:
            trace = False
    return run_gat(cfg, inputs, trace=trace).astype(np.float32)



# revision 14
# speedup vs baseline: 1.3915x; 1.0891x over previous
"""3-layer GAT (PyG GATConv semantics) forward on 8 Trainium2 NeuronCores.

Strategy (graph/data parallel, dst-sharded):
  - Nodes padded to N_PAD = 8*98*128 and sharded by destination across 8 cores.
  - Edges (plus self-loops) bucketed host-side by (core, dst-tile, src-bank),
    sorted by dst, padded to 128-edge chunks; chunk structure equalized across
    cores so one SPMD program serves all 8.
  - Per layer: each core computes x_aug = h @ [W | W*a_src | W*a_dst] for its
    node shard (x in bf16, attention logits in f32), AllGathers the packed
    row table, then processes its dst tiles: dma_gather (4 SWDGE queues)
    fetches x_aug rows by src, attention weights are computed with the
    exp(leaky_relu(al_src + al_dst)) folded per edge, and the per-dst softmax
    numerator/denominator are accumulated with one-hot matmuls on TensorE.
    al_dst is expanded dst->edges with a transposed one-hot matmul (hi/lo bf16
    split keeps f32 precision).
"""
import os
import numpy as np
import ml_dtypes

import concourse.bass as bass
import concourse.bacc as bacc
import concourse.tile as tile
import concourse.mybir as mybir
from concourse import ap_utils
from concourse.bass_utils import run_bass_kernel_spmd

F32 = mybir.dt.float32
BF16 = mybir.dt.bfloat16
U16 = mybir.dt.uint16
I16 = mybir.dt.int16
FP8 = mybir.dt.float8e4
OH_DT = FP8 if os.environ.get("GAT_FP8OH", "1") == "1" else BF16
AF = mybir.ActivationFunctionType
OP = mybir.AluOpType
P = 128
BF_NP = ml_dtypes.bfloat16

LAST_RESULT = {}


# ----------------------------------------------------------------------------
# configuration
# ----------------------------------------------------------------------------
class Cfg:
    def __init__(self, n_nodes=100000, tiles_per_core=98, bank_rows=25088,
                 ncores=8, heads=(8, 8, 1), ch=(32, 32, 40), fin0=128):
        self.n_nodes = n_nodes
        self.ncores = ncores
        self.tiles = tiles_per_core
        self.shard = tiles_per_core * P
        self.n_pad = ncores * self.shard
        assert self.n_pad >= n_nodes
        self.bank_rows = bank_rows
        assert bank_rows % P == 0 and bank_rows <= 32768
        self.nbanks = (self.n_pad + bank_rows - 1) // bank_rows
        self.heads = list(heads)
        self.ch = list(ch)
        self.fin = [fin0, heads[0] * ch[0], heads[1] * ch[1]]
        # per-layer u16 table geometry: x cols (bf16) | al_src f32 | al_dst f32
        self.geom = []
        for l in range(3):
            xc = self.heads[l] * self.ch[l]
            elem = xc + 2 * self.heads[l]          # u16: x bf16 + al_src f32
            stride = ((xc + 4 * self.heads[l]) + 127) // 128 * 128
            self.geom.append(dict(xc=xc, elem=elem, stride=stride,
                                  alsrc=xc, aldst=xc + 2 * self.heads[l]))


# ----------------------------------------------------------------------------
# host-side graph preprocessing
# ----------------------------------------------------------------------------
def _wrap_idx(flat):
    """flat[e] (e = c*128 + p) -> [128, n/16] int16 tile for dma_gather.
    HW mapping (measured): out[p, c] = table[idx_sbuf[p % 16, p//16 + 8*c]]."""
    n = len(flat)
    B = flat.reshape(n // 128, 8, 16).transpose(2, 0, 1).reshape(16, n // 16)
    return np.tile(B, (8, 1))


def preprocess(cfg, edge_index):
    src = np.concatenate([np.asarray(edge_index[0]),
                          np.arange(cfg.n_nodes, dtype=np.int64)]).astype(np.int64)
    dst = np.concatenate([np.asarray(edge_index[1]),
                          np.arange(cfg.n_nodes, dtype=np.int64)]).astype(np.int64)
    ne = len(src)
    core = dst // cfg.shard
    tile_ic = (dst % cfg.shard) // P
    bank = src // cfg.bank_rows
    dst_local = (dst % P).astype(np.int32)
    src_local = (src - bank * cfg.bank_rows).astype(np.int32)

    ngroups = cfg.ncores * cfg.tiles * cfg.nbanks
    key = ((core * cfg.tiles + tile_ic) * cfg.nbanks + bank).astype(np.int64)
    cnt = np.bincount(key, minlength=ngroups).reshape(cfg.ncores, cfg.tiles, cfg.nbanks)
    ch_tb = (cnt.max(axis=0) + P - 1) // P          # [tiles, nbanks] chunks, shared
    assert ch_tb.max() <= 8, f"gather call would exceed 1024 idxs: {ch_tb.max()}"
    ni_tb = ch_tb * P
    stream_len = int(ni_tb.sum())                   # per-core padded edge stream

    # static offsets of each (t, b) group in the padded stream (t-major)
    off_tb = np.zeros((cfg.tiles, cfg.nbanks), np.int64)
    acc = 0
    for t in range(cfg.tiles):
        for b in range(cfg.nbanks):
            off_tb[t, b] = acc
            acc += ni_tb[t, b]

    # scatter edges into the padded per-core streams
    order = np.argsort(key, kind="stable")
    key_s = key[order]
    group_start = np.zeros(ngroups + 1, np.int64)
    np.cumsum(np.bincount(key_s, minlength=ngroups), out=group_start[1:])
    pos_in_group = np.arange(ne, dtype=np.int64) - group_start[key_s]
    tb_flat = key_s % (cfg.tiles * cfg.nbanks)
    t_of = tb_flat // cfg.nbanks
    b_of = tb_flat % cfg.nbanks
    stream_pos = off_tb[t_of, b_of] + pos_in_group
    core_s = key_s // (cfg.tiles * cfg.nbanks)

    srcl_pad = np.zeros((cfg.ncores, stream_len), np.int32)      # pad -> row 0
    dstl_pad = np.full((cfg.ncores, stream_len), 200, np.int32)  # pad sentinel
    srcl_pad[core_s, stream_pos] = src_local[order]
    dstl_pad[core_s, stream_pos] = dst_local[order]

    # per-tile chunk geometry
    nch_t = ch_tb.sum(axis=1).astype(np.int64)                  # chunks per tile
    nch_max = int(nch_t.max())
    meta = dict(ch_tb=ch_tb, ni_tb=ni_tb, off_tb=off_tb, nch_t=nch_t,
                nch_max=nch_max, stream_len=stream_len)

    # per-core flat device arrays; per-tile idx blocks are [128, nch_t*8]
    # (per-bank wrapped blocks concatenated on axis 1 -> one DMA per tile)
    idx_flats, dc_flats, dr_flats = [], [], []
    idx_off = np.zeros(cfg.tiles, np.int64)          # u16 offset of tile block
    idx_boff = np.zeros((cfg.tiles, cfg.nbanks), np.int64)   # col offset /16
    dc_off = np.zeros(cfg.tiles, np.int64)
    dr_off = np.zeros(cfg.tiles, np.int64)
    for c in range(cfg.ncores):
        idx_parts, dc_parts, dr_parts = [], [], []
        ioff = 0
        for t in range(cfg.tiles):
            seg0 = int(off_tb[t, 0])
            nt = int(nch_t[t]) * P
            seg = slice(seg0, seg0 + nt)
            dl = dstl_pad[c, seg]
            if c == 0:
                dc_off[t] = sum(x.size for x in dc_parts)
                dr_off[t] = sum(x.size for x in dr_parts)
                idx_off[t] = ioff
            dc_parts.append(dl.reshape(-1, P).T.astype(BF_NP).ravel())
            dr_parts.append(dl.astype(BF_NP))
            blocks = []
            coloff = 0
            for b in range(cfg.nbanks):
                ni = int(ni_tb[t, b])
                if ni == 0:
                    continue
                if c == 0:
                    idx_boff[t, b] = coloff
                blocks.append(_wrap_idx(
                    srcl_pad[c, int(off_tb[t, b]):int(off_tb[t, b]) + ni]
                    .astype(np.int16)))
                coloff += ni // 16
            tile_idx = np.concatenate(blocks, axis=1)   # [128, nch_t*8]
            idx_parts.append(tile_idx.ravel())
            ioff += tile_idx.size
        idx_flats.append(np.concatenate(idx_parts))
        dc_flats.append(np.concatenate(dc_parts))
        dr_flats.append(np.concatenate(dr_parts))
    meta.update(idx_off=idx_off, idx_boff=idx_boff, dc_off=dc_off, dr_off=dr_off,
                idx_len=len(idx_flats[0]), dc_len=len(dc_flats[0]),
                dr_len=len(dr_flats[0]))
    return meta, idx_flats, dc_flats, dr_flats


def make_weights(cfg, inputs):
    """Per-layer: W_x bf16 [fin, xc]; W_al f32 [fin, 2H]; b_rep f32 [128, xc]."""
    out = {}
    for l in range(3):
        W = np.asarray(inputs[f"W{l}"], np.float32)
        a_src = np.asarray(inputs[f"a_src{l}"], np.float32)
        a_dst = np.asarray(inputs[f"a_dst{l}"], np.float32)
        b = np.asarray(inputs[f"b{l}"], np.float32)
        H, C = a_src.shape
        wal = np.zeros((W.shape[0], 2 * H), np.float32)
        for h in range(H):
            wal[:, h] = W[:, h * C:(h + 1) * C] @ a_src[h]
            wal[:, H + h] = W[:, h * C:(h + 1) * C] @ a_dst[h]
        out[f"wx{l}"] = W.astype(BF_NP)
        out[f"wal{l}"] = wal
        out[f"brep{l}"] = np.broadcast_to(b, (P, len(b))).copy()
    return out


# ----------------------------------------------------------------------------
# patched dma_gather (non-transpose HBM source; elem bytes need not be %256)
# ----------------------------------------------------------------------------
def dma_gather_unaligned(gpsimd, out_ap, in_ap, idxs_ap, num_idxs, elem_size,
                         elem_step, queue_num=0):
    assert idxs_ap.dtype == I16
    assert in_ap.dtype == out_ap.dtype
    assert ap_utils.ap_is_contiguous(in_ap.ap[1:])
    assert ap_utils.ap_is_contiguous(out_ap.ap[1:])
    assert ap_utils.ap_is_contiguous(idxs_ap.ap[1:])
    assert in_ap.ap[-1][1] == out_ap.ap[-1][1] == elem_size
    assert out_ap.ap[0][1] * out_ap.ap[1][1] == (num_idxs + 127) // 128 * 128
    assert in_ap.ap[0][0] == elem_step
    dtsz = mybir.dt.size(in_ap.dtype)
    stride_bytes = elem_step * dtsz
    assert stride_bytes % 256 == 0 and stride_bytes // 256 < 256
    _in_ap = gpsimd.lower_ap_dma(in_ap, for_custom_bir_dma=True)
    _idxs_ap = gpsimd.lower_ap(idxs_ap)
    _out_ap = gpsimd.lower_ap(out_ap)
    return gpsimd.add_instruction(
        mybir.InstDMAGatherAnt(
            name=gpsimd.bass.get_next_instruction_name(),
            ins=[*_in_ap, _idxs_ap,
                 gpsimd.lower_val_access(gpsimd.to_reg(num_idxs))],
            outs=[_out_ap],
            transpose=False, num_idxs=num_idxs, elem_size=elem_size,
            stride_bytes_256=stride_bytes // 256, gen_mode=0,
            single_packet=True, queue_num=queue_num,
            sbuf_tokens_per_rank=0, sbuf_free_dim_per_rank=0,
            sbuf_free_dim_pad_per_rank=0, sbuf_byte_offset=0,
        ))


# ----------------------------------------------------------------------------
# kernel builder
# ----------------------------------------------------------------------------
def build(cfg, meta):
    nc = bacc.Bacc("TRN2", target_bir_lowering=False, debug=False,
                   num_devices=cfg.ncores, num_swdge_queues=4,
                   dynamic_dma_scratch_size=32768)
    g0 = cfg.geom[0]

    feats = nc.dram_tensor("feats", [cfg.shard, cfg.fin[0]], F32, kind="ExternalInput")
    idxs = nc.dram_tensor("idxs", [meta["idx_len"]], I16, kind="ExternalInput")
    dcol = nc.dram_tensor("dcol", [meta["dc_len"]], BF16, kind="ExternalInput")
    drow = nc.dram_tensor("drow", [meta["dr_len"]], BF16, kind="ExternalInput")
    wx, wal, brep = [], [], []
    for l in range(3):
        wx.append(nc.dram_tensor(f"wx{l}", [cfg.fin[l], cfg.geom[l]["xc"]], BF16,
                                 kind="ExternalInput"))
        wal.append(nc.dram_tensor(f"wal{l}", [cfg.fin[l], 2 * cfg.heads[l]], F32,
                                  kind="ExternalInput"))
        brep.append(nc.dram_tensor(f"brep{l}", [P, cfg.geom[l]["xc"]], F32,
                                   kind="ExternalInput"))
    ident_in = nc.dram_tensor("ident", [P, P], F32, kind="ExternalInput")
    iota_in = nc.dram_tensor("iota", [P, P], BF16, kind="ExternalInput")
    iotac_in = nc.dram_tensor("iotac", [P, 1], BF16, kind="ExternalInput")
    iotacf_in = nc.dram_tensor("iotacf", [P, 1], F32, kind="ExternalInput")
    ones_in = nc.dram_tensor("ones", [1, P], BF16, kind="ExternalInput")
    out_sh = nc.dram_tensor("out_shard", [cfg.shard, cfg.ch[2]], F32,
                            kind="ExternalOutput")

    NQ = int(os.environ.get("GAT_NQ", "4"))
    ch_tb, ni_tb, nch_t = meta["ch_tb"], meta["ni_tb"], meta["nch_t"]
    idx_off, idx_boff = meta["idx_off"], meta["idx_boff"]
    dc_off, dr_off = meta["dc_off"], meta["dr_off"]
    NCH = meta["nch_max"]

    with tile.TileContext(nc) as tc:
        with (
            tc.tile_pool(name="const", bufs=1) as cp,
            tc.tile_pool(name="sb", bufs=2) as sp,
            tc.tile_pool(name="ps", bufs=1, space="PSUM") as pp,
            tc.tile_pool(name="dram", bufs=1, space="DRAM") as dp,
        ):
            # ---------------- constants ----------------
            ident = cp.tile([P, P], F32)
            nc.sync.dma_start(out=ident[:], in_=ident_in[:, :])
            iota = cp.tile([P, P], BF16)
            nc.sync.dma_start(out=iota[:], in_=iota_in[:, :])
            iotac = cp.tile([P, 1], BF16)
            nc.sync.dma_start(out=iotac[:], in_=iotac_in[:, :])
            iotacf = cp.tile([P, 1], F32)
            nc.sync.dma_start(out=iotacf[:], in_=iotacf_in[:, :])
            ones = cp.tile([1, P], BF16)
            nc.sync.dma_start(out=ones[:], in_=ones_in[:, :])
            wx_t, wal_t, b_t = [], [], []
            for l in range(3):
                nf = cfg.fin[l] // P
                t = cp.tile([P, nf, cfg.geom[l]["xc"]], BF16, name=f"wxt{l}")
                nc.sync.dma_start(
                    out=t[:], in_=wx[l].ap().rearrange("(f p) c -> p f c", p=P))
                wx_t.append(t)
                t = cp.tile([P, nf, 2 * cfg.heads[l]], F32, name=f"walt{l}")
                nc.sync.dma_start(
                    out=t[:], in_=wal[l].ap().rearrange("(f p) c -> p f c", p=P))
                wal_t.append(t)
                t = cp.tile([P, cfg.geom[l]["xc"]], F32, name=f"bt{l}")
                nc.sync.dma_start(out=t[:], in_=brep[l].ap())
                b_t.append(t)

            # ---------------- DRAM bounces ----------------
            agin, table = [], []
            for l in range(3):
                st = cfg.geom[l]["stride"]
                agin.append(dp.tile([cfg.shard, st], U16, name=f"agin{l}"))
                table.append(dp.tile([cfg.n_pad, st], U16, name=f"table{l}",
                                     addr_space="Shared"))

            # ---------------- helpers ----------------
            def phase_a(l, t, h_tile):
                """h_tile: [128, fin] f32 SBUF -> writes agin[l] rows of tile t."""
                g = cfg.geom[l]
                nf = cfg.fin[l] // P
                hT = sp.tile([P, nf, P], F32, tag="hT")
                hTb = sp.tile([P, nf, P], BF16, tag="hTb")
                for f in range(nf):
                    tp = pp.tile([P, P], F32, space="PSUM", tag="scr", bufs=2)
                    nc.tensor.transpose(out=tp[:], in_=h_tile[:, f * P:(f + 1) * P],
                                        identity=ident[:])
                    nc.vector.tensor_copy(out=hT[:, f, :], in_=tp[:])
                    nc.scalar.activation(out=hTb[:, f, :], in_=hT[:, f, :],
                                         func=AF.Copy)
                aps = pp.tile([P, g["xc"] + 2 * cfg.heads[l]], F32, space="PSUM",
                              tag="aps")
                for f in range(nf):
                    nc.tensor.matmul(out=aps[:, 0:g["xc"]], lhsT=hTb[:, f, :],
                                     rhs=wx_t[l][:, f, :],
                                     start=(f == 0), stop=(f == nf - 1))
                for f in range(nf):
                    nc.tensor.matmul(out=aps[:, g["xc"]:], lhsT=hT[:, f, :],
                                     rhs=wal_t[l][:, f, :],
                                     start=(f == 0), stop=(f == nf - 1))
                row = sp.tile([P, g["stride"]], U16, tag="row")
                pad0 = g["xc"] + 4 * cfg.heads[l]
                if pad0 < g["stride"]:
                    nc.vector.memset(row[:, pad0:g["stride"]], 0)
                rb = row[:].bitcast(BF16)
                nc.scalar.activation(out=rb[:, 0:g["xc"]], in_=aps[:, 0:g["xc"]],
                                     func=AF.Copy)
                rf = row[:].bitcast(F32)
                H_ = cfg.heads[l]
                nc.vector.tensor_copy(out=rf[:, g["xc"] // 2:g["xc"] // 2 + H_],
                                      in_=aps[:, g["xc"]:g["xc"] + H_])
                # al_dst as bf16 hi/lo pair at u16 cols [xc+2H : xc+4H]
                hi_sl = rb[:, g["xc"] + 2 * H_:g["xc"] + 3 * H_]
                nc.scalar.activation(out=hi_sl, in_=aps[:, g["xc"] + H_:], func=AF.Copy)
                nc.vector.tensor_tensor(out=rb[:, g["xc"] + 3 * H_:g["xc"] + 4 * H_],
                                        in0=aps[:, g["xc"] + H_:], in1=hi_sl,
                                        op=OP.subtract)
                nc.scalar.dma_start(out=agin[l][t * P:(t + 1) * P, :], in_=row[:])

            def edge_loads(l, t):
                """stage 0: idx/dcol/drow/aldr loads + 4-bank gathers."""
                g = cfg.geom[l]
                H = cfg.heads[l]
                xc = g["xc"]
                nch = int(nch_t[t])
                E = nch * P

                gt = sp.tile([P, nch, g["elem"]], U16, tag="g", bufs=3)
                it = sp.tile([P, nch * 8], I16, tag="idx", bufs=6)
                nc.sync.dma_start(
                    out=it[:],
                    in_=idxs.ap()[int(idx_off[t]):int(idx_off[t]) + P * nch * 8]
                    .rearrange("(p m) -> p m", p=P))
                coff = 0
                for b in range(cfg.nbanks):
                    chb = int(ch_tb[t, b])
                    if chb == 0:
                        continue
                    ni = chb * P
                    rows = min(cfg.bank_rows, cfg.n_pad - b * cfg.bank_rows)
                    dma_gather_unaligned(
                        nc.gpsimd,
                        out_ap=gt[:, coff:coff + chb, :],
                        in_ap=table[l][b * cfg.bank_rows:b * cfg.bank_rows + rows,
                                       0:g["elem"]],
                        idxs_ap=it[:, int(idx_boff[t, b]):int(idx_boff[t, b]) + ni // 16],
                        num_idxs=ni, elem_size=g["elem"],
                        elem_step=g["stride"], queue_num=b % NQ)
                    coff += chb

                dcol_t = sp.tile([P, nch], BF16, tag="dcol", bufs=6)
                nc.sync.dma_start(
                    out=dcol_t[:],
                    in_=dcol.ap()[int(dc_off[t]):int(dc_off[t]) + P * nch]
                    .rearrange("(p m) -> p m", p=P))
                drow_t = sp.tile([1, E], BF16, tag="drow", bufs=3)
                nc.sync.dma_start(
                    out=drow_t[:],
                    in_=drow.ap()[int(dr_off[t]):int(dr_off[t]) + E].unsqueeze(0))
                hilo = sp.tile([P, 2 * H], BF16, tag="hilo", bufs=6)
                nc.sync.dma_start(
                    out=hilo[:].bitcast(U16),
                    in_=agin[l][t * P:(t + 1) * P, xc + 2 * H:xc + 4 * H])
                return dict(gt=gt, dcol_t=dcol_t, drow_t=drow_t, hilo=hilo)

            def edge_front(l, t, ld):
                """one-hot builds + al_dst expansion (deps: loads of t only)."""
                g = cfg.geom[l]
                H = cfg.heads[l]
                xc = g["xc"]
                nch = int(nch_t[t])
                E = nch * P
                dcol_t, drow_t, hilo = ld["dcol_t"], ld["drow_t"], ld["hilo"]

                oh = sp.tile([P, nch, P], OH_DT, tag="oh", bufs=3)
                nc.vector.tensor_tensor(
                    out=oh[:],
                    in0=dcol_t[:].unsqueeze(2).to_broadcast([P, nch, P]),
                    in1=iota[:].unsqueeze(1).to_broadcast([P, nch, P]),
                    op=OP.is_equal)
                dstb = sp.tile([P, E], BF16, tag="dstb", bufs=3)
                for pi, s0 in enumerate(range(0, E, 512)):
                    s1 = min(s0 + 512, E)
                    bc = pp.tile([P, 512], F32, space="PSUM", tag="scr", bufs=2)
                    nc.tensor.matmul(out=bc[:, 0:s1 - s0], lhsT=ones[:],
                                     rhs=drow_t[:, s0:s1], start=True, stop=True)
                    nc.scalar.activation(out=dstb[:, s0:s1],
                                         in_=bc[:, 0:s1 - s0], func=AF.Copy)
                ohT = sp.tile([P, E], OH_DT, tag="ohT", bufs=3)
                nc.vector.tensor_scalar(out=ohT[:], in0=dstb[:],
                                        scalar1=iotacf[:, 0:1], scalar2=0.0,
                                        op0=OP.subtract, op1=OP.is_equal)
                adx = pp.tile([P, nch * 2 * H], F32, space="PSUM", tag="adx", bufs=2)
                for c in range(nch):
                    nc.tensor.matmul(out=adx[:, c * 2 * H:(c + 1) * 2 * H],
                                     lhsT=ohT[:, c * P:(c + 1) * P],
                                     rhs=hilo[:], start=True, stop=True)
                return dict(oh=oh, adx=adx)

            def edge_back(l, t, ld, fr):
                """attention weights + weighted values + segment sums."""
                g = cfg.geom[l]
                H = cfg.heads[l]
                C = cfg.ch[l]
                xc = g["xc"]
                nch = int(nch_t[t])
                gt, oh, adx = ld["gt"], fr["oh"], fr["adx"]

                gf = gt[:].bitcast(F32)
                alsrc = gf[:, :, xc // 2:xc // 2 + H]
                a2 = adx[:].rearrange("p (c k) -> p c k", k=2 * H)
                S = sp.tile([P, nch, H], F32, tag="S", bufs=3)
                nc.vector.tensor_tensor(out=S[:], in0=alsrc, in1=a2[:, :, 0:H],
                                        op=OP.add)
                nc.vector.tensor_tensor(out=S[:], in0=S[:], in1=a2[:, :, H:2 * H],
                                        op=OP.add)
                S2 = sp.tile([P, nch, H], F32, tag="S2", bufs=3)
                nc.vector.scalar_tensor_tensor(out=S2[:], in0=S[:], scalar=0.2,
                                               in1=S[:], op0=OP.mult, op1=OP.max)
                gb = gt[:].bitcast(BF16)
                v = sp.tile([P, nch, xc + H], BF16, tag="v", bufs=2)
                nc.scalar.activation(out=v[:, :, xc:xc + H], in_=S2[:],
                                     func=AF.Exp)
                wexp = sp.tile([P, nch, xc], BF16, tag="wexp", bufs=2)
                nc.scalar.activation(
                    out=wexp[:].rearrange("p c (h x) -> p c h x", h=H),
                    in_=S2[:].unsqueeze(3).to_broadcast([P, nch, H, C]),
                    func=AF.Exp)
                nc.vector.tensor_tensor(out=v[:, :, 0:xc], in0=gb[:, :, 0:xc],
                                        in1=wexp[:], op=OP.mult)
                ops = pp.tile([P, xc + H], F32, space="PSUM", tag="ops", bufs=2)
                for c in range(nch):
                    nc.tensor.matmul(out=ops[:], lhsT=oh[:, c, :],
                                     rhs=v[:, c, :],
                                     start=(c == 0), stop=(c == nch - 1))
                return ops

            def edge_epi(l, t, ops):
                """normalize + bias (+ ELU); returns h_next or writes out."""
                g = cfg.geom[l]
                H = cfg.heads[l]
                C = cfg.ch[l]
                xc = g["xc"]
                se = sp.tile([P, H], F32, tag="se", bufs=3)
                nc.vector.tensor_scalar_add(out=se[:], in0=ops[:, xc:xc + H],
                                            scalar1=1e-30)
                rs = sp.tile([P, H], F32, tag="rs", bufs=3)
                nc.vector.reciprocal(out=rs[:], in_=se[:])
                h1 = sp.tile([P, xc], F32, tag="h1", bufs=3)
                nc.vector.tensor_tensor(
                    out=h1[:].rearrange("p (h x) -> p h x", h=H),
                    in0=ops[:, 0:xc].rearrange("p (h x) -> p h x", h=H),
                    in1=rs[:].unsqueeze(2).to_broadcast([P, H, C]),
                    op=OP.mult)
                h2 = sp.tile([P, xc], F32, tag="h2", bufs=3)
                nc.vector.tensor_tensor(out=h2[:], in0=h1[:], in1=b_t[l][:],
                                        op=OP.add)
                if l == 2:
                    nc.scalar.dma_start(out=out_sh[t * P:(t + 1) * P, :], in_=h2[:])
                    return None
                # elu(x) = relu(x) + exp(x - relu(x)) - 1  (keeps min off DVE)
                r = sp.tile([P, xc], F32, tag="m", bufs=3)
                nc.scalar.activation(out=r[:], in_=h2[:], func=AF.Relu)
                t = sp.tile([P, xc], F32, tag="t", bufs=2)
                nc.vector.tensor_tensor(out=t[:], in0=h2[:], in1=r[:],
                                        op=OP.subtract)
                nc.scalar.activation(out=t[:], in_=t[:], func=AF.Exp)
                hn = sp.tile([P, xc], F32, tag="hn", bufs=3)
                nc.vector.scalar_tensor_tensor(out=hn[:], in0=t[:], scalar=-1.0,
                                               in1=r[:], op0=OP.add, op1=OP.add)
                return hn

            # ---------------- program ----------------
            rg = [list(range(cfg.ncores))]

            def edge_phase(l, next_l):
                lds, frs, opss, hns = {}, {}, {}, {}
                T = cfg.tiles
                for t in range(T + 3):
                    if t < T:
                        lds[t] = edge_loads(l, t)
                    if t - 1 >= 0 and t - 1 < T:
                        frs[t - 1] = edge_front(l, t - 1, lds[t - 1])
                    if t - 2 >= 0 and t - 2 < T:
                        u = t - 2
                        opss[u] = edge_back(l, u, lds.pop(u), frs.pop(u))
                    if t - 3 >= 0 and t - 3 < T:
                        u = t - 3
                        hn = edge_epi(l, u, opss.pop(u))
                        if next_l is not None:
                            phase_a(next_l, u, hn)

            # zero the gather pool so never-gathered pad slots hold finite 0s
            for _ in range(3):
                gz = sp.tile([P, NCH, cfg.geom[0]["elem"]], U16, tag="g", bufs=3)
                nc.vector.memset(gz[:], 0)

            # layer 0 phase A from features
            for t in range(cfg.tiles):
                h0 = sp.tile([P, cfg.fin[0]], F32, tag="h0")
                nc.sync.dma_start(out=h0[:], in_=feats.ap()[t * P:(t + 1) * P, :])
                phase_a(0, t, h0)
            nc.gpsimd.collective_compute(
                "AllGather", OP.bypass, ins=[agin[0][:].opt()],
                outs=[table[0][:].opt()], replica_groups=rg)
            edge_phase(0, 1)
            nc.gpsimd.collective_compute(
                "AllGather", OP.bypass, ins=[agin[1][:].opt()],
                outs=[table[1][:].opt()], replica_groups=rg)
            edge_phase(1, 2)
            nc.gpsimd.collective_compute(
                "AllGather", OP.bypass, ins=[agin[2][:].opt()],
                outs=[table[2][:].opt()], replica_groups=rg)
            edge_phase(2, None)

    nc.compile()
    return nc


# ----------------------------------------------------------------------------
# entry point
# ----------------------------------------------------------------------------
def run_gat(cfg, inputs, trace=False):
    meta, idx_flats, dc_flats, dr_flats = preprocess(cfg, inputs["edge_index"])
    wts = make_weights(cfg, inputs)
    feats = np.asarray(inputs["features"], np.float32)
    feats_pad = np.zeros((cfg.n_pad, cfg.fin[0]), np.float32)
    feats_pad[:cfg.n_nodes] = feats

    nc = build(cfg, meta)

    shared = dict(wts)
    shared["ident"] = np.eye(P, dtype=np.float32)
    shared["iota"] = np.broadcast_to(np.arange(P, dtype=np.float32), (P, P)).astype(BF_NP)
    shared["iotac"] = np.arange(P, dtype=np.float32).reshape(P, 1).astype(BF_NP)
    shared["iotacf"] = np.arange(P, dtype=np.float32).reshape(P, 1)
    shared["ones"] = np.ones((1, P), BF_NP)
    in_maps = []
    for c in range(cfg.ncores):
        m = dict(shared)
        m["feats"] = feats_pad[c * cfg.shard:(c + 1) * cfg.shard]
        m["idxs"] = idx_flats[c]
        m["dcol"] = dc_flats[c]
        m["drow"] = dr_flats[c]
        in_maps.append(m)

    res = run_bass_kernel_spmd(nc, in_maps, core_ids=list(range(cfg.ncores)),
                               trace=trace)
    LAST_RESULT["exec_time_ns"] = res.exec_time_ns
    out = np.concatenate([res.results[c]["out_shard"] for c in range(cfg.ncores)],
                         axis=0)[:cfg.n_nodes]
    return out


def kernel(**inputs):
    cfg = Cfg()
    trace = os.environ.get("GAT_TRACE", "0") == "1"
    if trace:
        try:
            import sys as _sys, types as _types
            import trn_agent_boot.trn_boot as _tb
            _m = _types.ModuleType("antenv.axon_hooks")
            _hook = _tb._ntff_profile_via_ctypes("/opt/axon/libaxon_pjrt.so")
            _m.get_axon_ntff_profile_hook = lambda: _hook
            _m.set_axon_ntff_profile_hook = lambda h: None
            _sys.modules.setdefault("antenv.axon_hooks", _m)
            import concourse.bass_utils as _bu
            _bu.upload_artifacts = lambda tmpdir: f"file://{tmpdir}"
        except Exception:
            trace = False
    return run_gat(cfg, inputs, trace=trace).astype(np.float32)



# revision 25
# speedup vs baseline: 1.4994x; 1.0775x over previous
"""3-layer GAT (PyG GATConv semantics) forward on 8 Trainium2 NeuronCores.

Strategy (graph/data parallel, dst-sharded):
  - Nodes padded to N_PAD = 8*98*128 and sharded by destination across 8 cores.
  - Edges (plus self-loops) bucketed host-side by (core, dst-tile, src-bank),
    sorted by dst, padded to 128-edge chunks; chunk structure equalized across
    cores so one SPMD program serves all 8.
  - Per layer: each core computes x_aug = h @ [W | W*a_src | W*a_dst] for its
    node shard (x in bf16, attention logits in f32), AllGathers the packed
    row table, then processes its dst tiles: dma_gather (4 SWDGE queues)
    fetches x_aug rows by src, attention weights are computed with the
    exp(leaky_relu(al_src + al_dst)) folded per edge, and the per-dst softmax
    numerator/denominator are accumulated with one-hot matmuls on TensorE.
    al_dst is expanded dst->edges with a transposed one-hot matmul (hi/lo bf16
    split keeps f32 precision).
"""
import os
import numpy as np
import ml_dtypes

import concourse.bass as bass
import concourse.bacc as bacc
import concourse.tile as tile
import concourse.mybir as mybir
from concourse import ap_utils
from concourse.bass_utils import run_bass_kernel_spmd

F32 = mybir.dt.float32
BF16 = mybir.dt.bfloat16
U16 = mybir.dt.uint16
I16 = mybir.dt.int16
FP8 = mybir.dt.float8e4
OH_DT = FP8 if os.environ.get("GAT_FP8OH", "1") == "1" else BF16
AF = mybir.ActivationFunctionType
OP = mybir.AluOpType
P = 128
BF_NP = ml_dtypes.bfloat16

LAST_RESULT = {}


# ----------------------------------------------------------------------------
# configuration
# ----------------------------------------------------------------------------
class Cfg:
    def __init__(self, n_nodes=100000, tiles_per_core=98, bank_rows=25088,
                 ncores=8, heads=(8, 8, 1), ch=(32, 32, 40), fin0=128):
        self.n_nodes = n_nodes
        self.ncores = ncores
        self.tiles = tiles_per_core
        self.shard = tiles_per_core * P
        self.n_pad = ncores * self.shard
        assert self.n_pad >= n_nodes
        self.bank_rows = bank_rows
        assert bank_rows % P == 0 and bank_rows <= 32768
        self.nbanks = (self.n_pad + bank_rows - 1) // bank_rows
        self.heads = list(heads)
        self.ch = list(ch)
        self.fin = [fin0, heads[0] * ch[0], heads[1] * ch[1]]
        # per-layer u16 table geometry: x cols (bf16) | al_src f32 | al_dst f32
        self.geom = []
        for l in range(3):
            xc = self.heads[l] * self.ch[l]
            elem = xc + 2 * self.heads[l]          # u16: x bf16 + al_src f32
            stride = ((xc + 4 * self.heads[l]) + 127) // 128 * 128
            self.geom.append(dict(xc=xc, elem=elem, stride=stride,
                                  alsrc=xc, aldst=xc + 2 * self.heads[l]))


# ----------------------------------------------------------------------------
# host-side graph preprocessing
# ----------------------------------------------------------------------------
def _wrap_idx(flat):
    """flat[e] (e = c*128 + p) -> [128, n/16] int16 tile for dma_gather.
    HW mapping (measured): out[p, c] = table[idx_sbuf[p % 16, p//16 + 8*c]]."""
    n = len(flat)
    B = flat.reshape(n // 128, 8, 16).transpose(2, 0, 1).reshape(16, n // 16)
    return np.tile(B, (8, 1))


def preprocess(cfg, edge_index):
    src = np.concatenate([np.asarray(edge_index[0]),
                          np.arange(cfg.n_nodes, dtype=np.int64)]).astype(np.int64)
    dst = np.concatenate([np.asarray(edge_index[1]),
                          np.arange(cfg.n_nodes, dtype=np.int64)]).astype(np.int64)
    ne = len(src)
    core = dst // cfg.shard
    tile_ic = (dst % cfg.shard) // P
    # permuted src id: half-shard-major so the AllGather of each half-shard
    # lands contiguously (half 0 = banks 0,1; half 1 = banks 2,3)
    half = cfg.shard // 2
    c_of = src // cfg.shard
    r_of = src % cfg.shard
    h_of = (r_of >= half).astype(np.int64)
    psrc = h_of * (cfg.ncores * half) + c_of * half + (r_of - h_of * half)
    bank = psrc // cfg.bank_rows
    dst_local = (dst % P).astype(np.int32)
    src_local = (psrc - bank * cfg.bank_rows).astype(np.int32)

    ngroups = cfg.ncores * cfg.tiles * cfg.nbanks
    key = ((core * cfg.tiles + tile_ic) * cfg.nbanks + bank).astype(np.int64)
    cnt = np.bincount(key, minlength=ngroups).reshape(cfg.ncores, cfg.tiles, cfg.nbanks)
    ch_tb = (cnt.max(axis=0) + P - 1) // P          # [tiles, nbanks] chunks, shared
    # exact gather count (max across cores, 16-aligned): skips most pad slots
    ni16_tb = np.maximum((cnt.max(axis=0) + 15) // 16 * 16, 16)
    assert ch_tb.max() <= 8, f"gather call would exceed 1024 idxs: {ch_tb.max()}"
    ni_tb = ch_tb * P
    stream_len = int(ni_tb.sum())                   # per-core padded edge stream

    # static offsets of each (t, b) group in the padded stream (t-major)
    off_tb = np.zeros((cfg.tiles, cfg.nbanks), np.int64)
    acc = 0
    for t in range(cfg.tiles):
        for b in range(cfg.nbanks):
            off_tb[t, b] = acc
            acc += ni_tb[t, b]

    # scatter edges into the padded per-core streams
    order = np.argsort(key, kind="stable")
    key_s = key[order]
    group_start = np.zeros(ngroups + 1, np.int64)
    np.cumsum(np.bincount(key_s, minlength=ngroups), out=group_start[1:])
    pos_in_group = np.arange(ne, dtype=np.int64) - group_start[key_s]
    tb_flat = key_s % (cfg.tiles * cfg.nbanks)
    t_of = tb_flat // cfg.nbanks
    b_of = tb_flat % cfg.nbanks
    stream_pos = off_tb[t_of, b_of] + pos_in_group
    core_s = key_s // (cfg.tiles * cfg.nbanks)

    srcl_pad = np.zeros((cfg.ncores, stream_len), np.int32)      # pad -> row 0
    dstl_pad = np.full((cfg.ncores, stream_len), 200, np.int32)  # pad sentinel
    srcl_pad[core_s, stream_pos] = src_local[order]
    dstl_pad[core_s, stream_pos] = dst_local[order]

    # per-tile chunk geometry
    nch_t = ch_tb.sum(axis=1).astype(np.int64)                  # chunks per tile
    nch_max = int(nch_t.max())
    meta = dict(ch_tb=ch_tb, ni_tb=ni_tb, ni16_tb=ni16_tb, off_tb=off_tb,
                nch_t=nch_t, nch_max=nch_max, stream_len=stream_len)

    # per-core flat device arrays; per-tile idx blocks are [128, nch_t*8]
    # (per-bank wrapped blocks concatenated on axis 1 -> one DMA per tile)
    idx_flats, dc_flats, dr_flats = [], [], []
    idx_off = np.zeros(cfg.tiles, np.int64)          # u16 offset of tile block
    idx_boff = np.zeros((cfg.tiles, cfg.nbanks), np.int64)   # col offset /16
    dc_off = np.zeros(cfg.tiles, np.int64)
    dr_off = np.zeros(cfg.tiles, np.int64)
    for c in range(cfg.ncores):
        idx_parts, dc_parts, dr_parts = [], [], []
        ioff = 0
        for t in range(cfg.tiles):
            seg0 = int(off_tb[t, 0])
            nt = int(nch_t[t]) * P
            seg = slice(seg0, seg0 + nt)
            dl = dstl_pad[c, seg]
            if c == 0:
                dc_off[t] = sum(x.size for x in dc_parts)
                dr_off[t] = sum(x.size for x in dr_parts)
                idx_off[t] = ioff
            dc_parts.append(dl.reshape(-1, P).T.astype(BF_NP).ravel())
            dr_parts.append(dl.astype(BF_NP))
            blocks = []
            coloff = 0
            for b in range(cfg.nbanks):
                ni = int(ni_tb[t, b])
                if ni == 0:
                    continue
                if c == 0:
                    idx_boff[t, b] = coloff
                blocks.append(_wrap_idx(
                    srcl_pad[c, int(off_tb[t, b]):int(off_tb[t, b]) + ni]
                    .astype(np.int16)))
                coloff += ni // 16
            tile_idx = np.concatenate(blocks, axis=1)   # [128, nch_t*8]
            idx_parts.append(tile_idx.ravel())
            ioff += tile_idx.size
        idx_flats.append(np.concatenate(idx_parts))
        dc_flats.append(np.concatenate(dc_parts))
        dr_flats.append(np.concatenate(dr_parts))
    meta.update(idx_off=idx_off, idx_boff=idx_boff, dc_off=dc_off, dr_off=dr_off,
                idx_len=len(idx_flats[0]), dc_len=len(dc_flats[0]),
                dr_len=len(dr_flats[0]))
    return meta, idx_flats, dc_flats, dr_flats


def make_weights(cfg, inputs):
    """Per-layer: W_x bf16 [fin, xc]; W_al f32 [fin, 2H]; b_rep f32 [128, xc]."""
    out = {}
    for l in range(3):
        W = np.asarray(inputs[f"W{l}"], np.float32)
        a_src = np.asarray(inputs[f"a_src{l}"], np.float32)
        a_dst = np.asarray(inputs[f"a_dst{l}"], np.float32)
        b = np.asarray(inputs[f"b{l}"], np.float32)
        H, C = a_src.shape
        wal = np.zeros((W.shape[0], 2 * H), np.float32)
        for h in range(H):
            wal[:, h] = W[:, h * C:(h + 1) * C] @ a_src[h]
            wal[:, H + h] = W[:, h * C:(h + 1) * C] @ a_dst[h]
        out[f"wx{l}"] = W.astype(BF_NP)
        out[f"wal{l}"] = wal
        out[f"brep{l}"] = np.broadcast_to(b, (P, len(b))).copy()
    return out


# ----------------------------------------------------------------------------
# patched dma_gather (non-transpose HBM source; elem bytes need not be %256)
# ----------------------------------------------------------------------------
def dma_gather_unaligned(gpsimd, out_ap, in_ap, idxs_ap, num_idxs, elem_size,
                         elem_step, queue_num=0):
    assert idxs_ap.dtype == I16
    assert in_ap.dtype == out_ap.dtype
    assert ap_utils.ap_is_contiguous(in_ap.ap[1:])
    assert ap_utils.ap_is_contiguous(out_ap.ap[1:])
    assert ap_utils.ap_is_contiguous(idxs_ap.ap[1:])
    assert in_ap.ap[-1][1] == out_ap.ap[-1][1] == elem_size
    assert out_ap.ap[0][1] * out_ap.ap[1][1] == (num_idxs + 127) // 128 * 128
    assert in_ap.ap[0][0] == elem_step
    dtsz = mybir.dt.size(in_ap.dtype)
    stride_bytes = elem_step * dtsz
    assert stride_bytes % 256 == 0 and stride_bytes // 256 < 256
    _in_ap = gpsimd.lower_ap_dma(in_ap, for_custom_bir_dma=True)
    _idxs_ap = gpsimd.lower_ap(idxs_ap)
    _out_ap = gpsimd.lower_ap(out_ap)
    return gpsimd.add_instruction(
        mybir.InstDMAGatherAnt(
            name=gpsimd.bass.get_next_instruction_name(),
            ins=[*_in_ap, _idxs_ap,
                 gpsimd.lower_val_access(gpsimd.to_reg(num_idxs))],
            outs=[_out_ap],
            transpose=False, num_idxs=num_idxs, elem_size=elem_size,
            stride_bytes_256=stride_bytes // 256, gen_mode=0,
            single_packet=True, queue_num=queue_num,
            sbuf_tokens_per_rank=0, sbuf_free_dim_per_rank=0,
            sbuf_free_dim_pad_per_rank=0, sbuf_byte_offset=0,
        ))


# ----------------------------------------------------------------------------
# kernel builder
# ----------------------------------------------------------------------------
def build(cfg, meta):
    nc = bacc.Bacc("TRN2", target_bir_lowering=False, debug=False,
                   num_devices=cfg.ncores, num_swdge_queues=4,
                   dynamic_dma_scratch_size=32768)
    g0 = cfg.geom[0]

    feats = nc.dram_tensor("feats", [cfg.shard, cfg.fin[0]], F32, kind="ExternalInput")
    idxs = nc.dram_tensor("idxs", [meta["idx_len"]], I16, kind="ExternalInput")
    dcol = nc.dram_tensor("dcol", [meta["dc_len"]], BF16, kind="ExternalInput")
    drow = nc.dram_tensor("drow", [meta["dr_len"]], BF16, kind="ExternalInput")
    wx, wal, brep = [], [], []
    for l in range(3):
        wx.append(nc.dram_tensor(f"wx{l}", [cfg.fin[l], cfg.geom[l]["xc"]], BF16,
                                 kind="ExternalInput"))
        wal.append(nc.dram_tensor(f"wal{l}", [cfg.fin[l], 2 * cfg.heads[l]], F32,
                                  kind="ExternalInput"))
        brep.append(nc.dram_tensor(f"brep{l}", [P, cfg.geom[l]["xc"]], F32,
                                   kind="ExternalInput"))
    ident_in = nc.dram_tensor("ident", [P, P], F32, kind="ExternalInput")
    iota_in = nc.dram_tensor("iota", [P, P], BF16, kind="ExternalInput")
    iotac_in = nc.dram_tensor("iotac", [P, 1], BF16, kind="ExternalInput")
    iotacf_in = nc.dram_tensor("iotacf", [P, 1], F32, kind="ExternalInput")
    ones_in = nc.dram_tensor("ones", [1, P], BF16, kind="ExternalInput")
    out_sh = nc.dram_tensor("out_shard", [cfg.shard, cfg.ch[2]], F32,
                            kind="ExternalOutput")

    NQ = int(os.environ.get("GAT_NQ", "4"))
    ch_tb, ni_tb, nch_t = meta["ch_tb"], meta["ni_tb"], meta["nch_t"]
    ni16_tb = meta["ni16_tb"]
    idx_off, idx_boff = meta["idx_off"], meta["idx_boff"]
    dc_off, dr_off = meta["dc_off"], meta["dr_off"]
    NCH = meta["nch_max"]

    with tile.TileContext(nc) as tc:
        with (
            tc.tile_pool(name="const", bufs=1) as cp,
            tc.tile_pool(name="sb", bufs=2) as sp,
            tc.tile_pool(name="ps", bufs=1, space="PSUM") as pp,
            tc.tile_pool(name="dram", bufs=1, space="DRAM") as dp,
        ):
            # ---------------- constants ----------------
            ident = cp.tile([P, P], F32)
            nc.sync.dma_start(out=ident[:], in_=ident_in[:, :])
            iota = cp.tile([P, P], BF16)
            nc.sync.dma_start(out=iota[:], in_=iota_in[:, :])
            iotac = cp.tile([P, 1], BF16)
            nc.sync.dma_start(out=iotac[:], in_=iotac_in[:, :])
            iotacf = cp.tile([P, 1], F32)
            nc.sync.dma_start(out=iotacf[:], in_=iotacf_in[:, :])
            ones = cp.tile([1, P], BF16)
            nc.sync.dma_start(out=ones[:], in_=ones_in[:, :])
            wx_t, wal_t, b_t = [], [], []
            for l in range(3):
                nf = cfg.fin[l] // P
                t = cp.tile([P, nf, cfg.geom[l]["xc"]], BF16, name=f"wxt{l}")
                nc.sync.dma_start(
                    out=t[:], in_=wx[l].ap().rearrange("(f p) c -> p f c", p=P))
                wx_t.append(t)
                t = cp.tile([P, nf, 2 * cfg.heads[l]], F32, name=f"walt{l}")
                nc.sync.dma_start(
                    out=t[:], in_=wal[l].ap().rearrange("(f p) c -> p f c", p=P))
                wal_t.append(t)
                t = cp.tile([P, cfg.geom[l]["xc"]], F32, name=f"bt{l}")
                nc.sync.dma_start(out=t[:], in_=brep[l].ap())
                b_t.append(t)

            # ---------------- DRAM bounces ----------------
            agin, table = [], []
            for l in range(3):
                st = cfg.geom[l]["stride"]
                agin.append(dp.tile([cfg.shard, st], U16, name=f"agin{l}"))
                table.append([dp.tile([cfg.n_pad // 2, st], U16,
                                      name=f"table{l}h{h}", addr_space="Shared")
                              for h in range(2)])

            # ---------------- helpers ----------------
            def phase_a(l, t, h_tile):
                """h_tile: [128, fin] f32 SBUF -> writes agin[l] rows of tile t."""
                g = cfg.geom[l]
                nf = cfg.fin[l] // P
                hT = sp.tile([P, nf, P], F32, tag="hT")
                hTb = sp.tile([P, nf, P], BF16, tag="hTb")
                for f in range(nf):
                    tp = pp.tile([P, P], F32, space="PSUM", tag="scr", bufs=2)
                    nc.tensor.transpose(out=tp[:], in_=h_tile[:, f * P:(f + 1) * P],
                                        identity=ident[:])
                    nc.vector.tensor_copy(out=hT[:, f, :], in_=tp[:])
                    nc.scalar.activation(out=hTb[:, f, :], in_=hT[:, f, :],
                                         func=AF.Copy)
                aps = pp.tile([P, g["xc"] + 2 * cfg.heads[l]], F32, space="PSUM",
                              tag="aps")
                for f in range(nf):
                    nc.tensor.matmul(out=aps[:, 0:g["xc"]], lhsT=hTb[:, f, :],
                                     rhs=wx_t[l][:, f, :],
                                     start=(f == 0), stop=(f == nf - 1))
                for f in range(nf):
                    nc.tensor.matmul(out=aps[:, g["xc"]:], lhsT=hT[:, f, :],
                                     rhs=wal_t[l][:, f, :],
                                     start=(f == 0), stop=(f == nf - 1))
                row = sp.tile([P, g["stride"]], U16, tag="row")
                pad0 = g["xc"] + 4 * cfg.heads[l]
                if pad0 < g["stride"]:
                    nc.vector.memset(row[:, pad0:g["stride"]], 0)
                rb = row[:].bitcast(BF16)
                nc.scalar.activation(out=rb[:, 0:g["xc"]], in_=aps[:, 0:g["xc"]],
                                     func=AF.Copy)
                rf = row[:].bitcast(F32)
                H_ = cfg.heads[l]
                nc.vector.tensor_copy(out=rf[:, g["xc"] // 2:g["xc"] // 2 + H_],
                                      in_=aps[:, g["xc"]:g["xc"] + H_])
                # al_dst as bf16 hi/lo pair at u16 cols [xc+2H : xc+4H]
                hi_sl = rb[:, g["xc"] + 2 * H_:g["xc"] + 3 * H_]
                nc.scalar.activation(out=hi_sl, in_=aps[:, g["xc"] + H_:], func=AF.Copy)
                nc.vector.tensor_tensor(out=rb[:, g["xc"] + 3 * H_:g["xc"] + 4 * H_],
                                        in0=aps[:, g["xc"] + H_:], in1=hi_sl,
                                        op=OP.subtract)
                nc.scalar.dma_start(out=agin[l][t * P:(t + 1) * P, :], in_=row[:])

            def edge_loads(l, t):
                """stage 0: idx/dcol/drow/aldr loads + 4-bank gathers."""
                g = cfg.geom[l]
                H = cfg.heads[l]
                xc = g["xc"]
                nch = int(nch_t[t])
                E = nch * P

                gt = sp.tile([P, nch, g["elem"]], U16, tag="g", bufs=3)
                it = sp.tile([P, nch * 8], I16, tag="idx", bufs=6)
                nc.sync.dma_start(
                    out=it[:],
                    in_=idxs.ap()[int(idx_off[t]):int(idx_off[t]) + P * nch * 8]
                    .rearrange("(p m) -> p m", p=P))
                coff = 0
                for b in range(cfg.nbanks):
                    chb = int(ch_tb[t, b])
                    if chb == 0:
                        continue
                    ni = chb * P
                    rows = min(cfg.bank_rows, cfg.n_pad - b * cfg.bank_rows)
                    dma_gather_unaligned(
                        nc.gpsimd,
                        out_ap=gt[:, coff:coff + chb, :],
                        in_ap=table[l][b // 2][
                            (b % 2) * cfg.bank_rows:(b % 2) * cfg.bank_rows + rows,
                            0:g["elem"]],
                        idxs_ap=it[:, int(idx_boff[t, b]):int(idx_boff[t, b]) + ni // 16],
                        num_idxs=int(ni16_tb[t, b]), elem_size=g["elem"],
                        elem_step=g["stride"], queue_num=b % NQ)
                    coff += chb

                dcol_t = sp.tile([P, nch], BF16, tag="dcol", bufs=6)
                nc.sync.dma_start(
                    out=dcol_t[:],
                    in_=dcol.ap()[int(dc_off[t]):int(dc_off[t]) + P * nch]
                    .rearrange("(p m) -> p m", p=P))
                drow_t = sp.tile([1, E], BF16, tag="drow", bufs=3)
                nc.sync.dma_start(
                    out=drow_t[:],
                    in_=drow.ap()[int(dr_off[t]):int(dr_off[t]) + E].unsqueeze(0))
                hilo = sp.tile([P, 2 * H], BF16, tag="hilo", bufs=6)
                nc.sync.dma_start(
                    out=hilo[:].bitcast(U16),
                    in_=agin[l][t * P:(t + 1) * P, xc + 2 * H:xc + 4 * H])
                return dict(gt=gt, dcol_t=dcol_t, drow_t=drow_t, hilo=hilo)

            def edge_front(l, t, ld):
                """one-hot builds + al_dst expansion (deps: loads of t only)."""
                g = cfg.geom[l]
                H = cfg.heads[l]
                xc = g["xc"]
                nch = int(nch_t[t])
                E = nch * P
                dcol_t, drow_t, hilo = ld["dcol_t"], ld["drow_t"], ld["hilo"]

                oh = sp.tile([P, nch, P], OH_DT, tag="oh", bufs=3)
                nc.vector.tensor_tensor(
                    out=oh[:],
                    in0=dcol_t[:].unsqueeze(2).to_broadcast([P, nch, P]),
                    in1=iota[:].unsqueeze(1).to_broadcast([P, nch, P]),
                    op=OP.is_equal)
                dstb = sp.tile([P, E], BF16, tag="dstb", bufs=3)
                for pi, s0 in enumerate(range(0, E, 512)):
                    s1 = min(s0 + 512, E)
                    bc = pp.tile([P, 512], F32, space="PSUM", tag="scr", bufs=2)
                    nc.tensor.matmul(out=bc[:, 0:s1 - s0], lhsT=ones[:],
                                     rhs=drow_t[:, s0:s1], start=True, stop=True)
                    nc.scalar.activation(out=dstb[:, s0:s1],
                                         in_=bc[:, 0:s1 - s0], func=AF.Copy)
                ohT = sp.tile([P, E], OH_DT, tag="ohT", bufs=3)
                nc.vector.tensor_scalar(out=ohT[:], in0=dstb[:],
                                        scalar1=iotacf[:, 0:1], scalar2=0.0,
                                        op0=OP.subtract, op1=OP.is_equal)
                adx = pp.tile([P, nch * 2 * H], F32, space="PSUM", tag="adx", bufs=2)
                for c in range(nch):
                    nc.tensor.matmul(out=adx[:, c * 2 * H:(c + 1) * 2 * H],
                                     lhsT=ohT[:, c * P:(c + 1) * P],
                                     rhs=hilo[:], start=True, stop=True)
                return dict(oh=oh, adx=adx)

            def edge_back(l, t, ld, fr):
                """attention weights + weighted values + segment sums."""
                g = cfg.geom[l]
                H = cfg.heads[l]
                C = cfg.ch[l]
                xc = g["xc"]
                nch = int(nch_t[t])
                gt, oh, adx = ld["gt"], fr["oh"], fr["adx"]

                gf = gt[:].bitcast(F32)
                alsrc = gf[:, :, xc // 2:xc // 2 + H]
                a2 = adx[:].rearrange("p (c k) -> p c k", k=2 * H)
                S = sp.tile([P, nch, H], F32, tag="S", bufs=3)
                nc.vector.tensor_tensor(out=S[:], in0=alsrc, in1=a2[:, :, 0:H],
                                        op=OP.add)
                nc.vector.tensor_tensor(out=S[:], in0=S[:], in1=a2[:, :, H:2 * H],
                                        op=OP.add)
                S2 = sp.tile([P, nch, H], F32, tag="S2", bufs=3)
                nc.vector.scalar_tensor_tensor(out=S2[:], in0=S[:], scalar=0.2,
                                               in1=S[:], op0=OP.mult, op1=OP.max)
                gb = gt[:].bitcast(BF16)
                v = sp.tile([P, nch, xc + H], BF16, tag="v", bufs=2)
                nc.scalar.activation(out=v[:, :, xc:xc + H], in_=S2[:],
                                     func=AF.Exp)
                wexp = sp.tile([P, nch, xc], BF16, tag="wexp", bufs=2)
                nc.scalar.activation(
                    out=wexp[:].rearrange("p c (h x) -> p c h x", h=H),
                    in_=S2[:].unsqueeze(3).to_broadcast([P, nch, H, C]),
                    func=AF.Exp)
                nc.vector.tensor_tensor(out=v[:, :, 0:xc], in0=gb[:, :, 0:xc],
                                        in1=wexp[:], op=OP.mult)
                ops = pp.tile([P, xc + H], F32, space="PSUM", tag="ops", bufs=2)
                for c in range(nch):
                    nc.tensor.matmul(out=ops[:], lhsT=oh[:, c, :],
                                     rhs=v[:, c, :],
                                     start=(c == 0), stop=(c == nch - 1))
                return ops

            def edge_epi(l, t, ops):
                """normalize + bias (+ ELU); returns h_next or writes out."""
                g = cfg.geom[l]
                H = cfg.heads[l]
                C = cfg.ch[l]
                xc = g["xc"]
                se = sp.tile([P, H], F32, tag="se", bufs=3)
                nc.vector.tensor_scalar_add(out=se[:], in0=ops[:, xc:xc + H],
                                            scalar1=1e-30)
                rs = sp.tile([P, H], F32, tag="rs", bufs=3)
                nc.vector.reciprocal(out=rs[:], in_=se[:])
                h1 = sp.tile([P, xc], F32, tag="h1", bufs=3)
                nc.vector.tensor_tensor(
                    out=h1[:].rearrange("p (h x) -> p h x", h=H),
                    in0=ops[:, 0:xc].rearrange("p (h x) -> p h x", h=H),
                    in1=rs[:].unsqueeze(2).to_broadcast([P, H, C]),
                    op=OP.mult)
                h2 = sp.tile([P, xc], F32, tag="h2", bufs=3)
                nc.vector.tensor_tensor(out=h2[:], in0=h1[:], in1=b_t[l][:],
                                        op=OP.add)
                if l == 2:
                    nc.scalar.dma_start(out=out_sh[t * P:(t + 1) * P, :], in_=h2[:])
                    return None
                # elu(x) = relu(x) + exp(x - relu(x)) - 1  (keeps min off DVE)
                r = sp.tile([P, xc], F32, tag="m", bufs=3)
                nc.scalar.activation(out=r[:], in_=h2[:], func=AF.Relu)
                t = sp.tile([P, xc], F32, tag="t", bufs=2)
                nc.vector.tensor_tensor(out=t[:], in0=h2[:], in1=r[:],
                                        op=OP.subtract)
                nc.scalar.activation(out=t[:], in_=t[:], func=AF.Exp)
                hn = sp.tile([P, xc], F32, tag="hn", bufs=3)
                nc.vector.scalar_tensor_tensor(out=hn[:], in0=t[:], scalar=-1.0,
                                               in1=r[:], op0=OP.add, op1=OP.add)
                return hn

            # ---------------- program ----------------
            rg = [list(range(cfg.ncores))]
            HS = cfg.shard // 2          # 49 tiles * 128 rows
            TS = cfg.n_pad // 2

            def ag_half(l, h):
                """AllGather one half-shard; half h lands at table rows
                [h*TS : (h+1)*TS] thanks to the psrc permutation."""
                nc.gpsimd.collective_compute(
                    "AllGather", OP.bypass,
                    ins=[agin[l][h * HS:(h + 1) * HS, :].opt()],
                    outs=[table[l][h][:].opt()],
                    replica_groups=rg)

            def edge_phase(l, next_l):
                lds, frs, opss, hns = {}, {}, {}, {}
                T = cfg.tiles
                for t in range(T + 3):
                    if t < T:
                        lds[t] = edge_loads(l, t)
                    if t - 1 >= 0 and t - 1 < T:
                        frs[t - 1] = edge_front(l, t - 1, lds[t - 1])
                    if t - 2 >= 0 and t - 2 < T:
                        u = t - 2
                        opss[u] = edge_back(l, u, lds.pop(u), frs.pop(u))
                    if t - 3 >= 0 and t - 3 < T:
                        u = t - 3
                        hn = edge_epi(l, u, opss.pop(u))
                        if next_l is not None:
                            phase_a(next_l, u, hn)
                            if u == HS // P - 1:
                                ag_half(next_l, 0)
                if next_l is not None:
                    ag_half(next_l, 1)

            # zero the gather pool so never-gathered pad slots hold finite 0s
            for _ in range(3):
                gz = sp.tile([P, NCH, cfg.geom[0]["elem"]], U16, tag="g", bufs=3)
                nc.vector.memset(gz[:], 0)

            # layer 0 phase A from features
            for t in range(cfg.tiles):
                h0 = sp.tile([P, cfg.fin[0]], F32, tag="h0")
                nc.sync.dma_start(out=h0[:], in_=feats.ap()[t * P:(t + 1) * P, :])
                phase_a(0, t, h0)
                if t == HS // P - 1:
                    ag_half(0, 0)
            ag_half(0, 1)
            edge_phase(0, 1)
            edge_phase(1, 2)
            edge_phase(2, None)

    nc.compile()
    return nc


# ----------------------------------------------------------------------------
# entry point
# ----------------------------------------------------------------------------
def run_gat(cfg, inputs, trace=False):
    meta, idx_flats, dc_flats, dr_flats = preprocess(cfg, inputs["edge_index"])
    wts = make_weights(cfg, inputs)
    feats = np.asarray(inputs["features"], np.float32)
    feats_pad = np.zeros((cfg.n_pad, cfg.fin[0]), np.float32)
    feats_pad[:cfg.n_nodes] = feats

    nc = build(cfg, meta)

    shared = dict(wts)
    shared["ident"] = np.eye(P, dtype=np.float32)
    shared["iota"] = np.broadcast_to(np.arange(P, dtype=np.float32), (P, P)).astype(BF_NP)
    shared["iotac"] = np.arange(P, dtype=np.float32).reshape(P, 1).astype(BF_NP)
    shared["iotacf"] = np.arange(P, dtype=np.float32).reshape(P, 1)
    shared["ones"] = np.ones((1, P), BF_NP)
    in_maps = []
    for c in range(cfg.ncores):
        m = dict(shared)
        m["feats"] = feats_pad[c * cfg.shard:(c + 1) * cfg.shard]
        m["idxs"] = idx_flats[c]
        m["dcol"] = dc_flats[c]
        m["drow"] = dr_flats[c]
        in_maps.append(m)

    res = run_bass_kernel_spmd(nc, in_maps, core_ids=list(range(cfg.ncores)),
                               trace=trace)
    LAST_RESULT["exec_time_ns"] = res.exec_time_ns
    out = np.concatenate([res.results[c]["out_shard"] for c in range(cfg.ncores)],
                         axis=0)[:cfg.n_nodes]
    return out


def kernel(**inputs):
    cfg = Cfg()
    trace = os.environ.get("GAT_TRACE", "0") == "1"
    if trace:
        try:
            import sys as _sys, types as _types
            import trn_agent_boot.trn_boot as _tb
            _m = _types.ModuleType("antenv.axon_hooks")
            _hook = _tb._ntff_profile_via_ctypes("/opt/axon/libaxon_pjrt.so")
            _m.get_axon_ntff_profile_hook = lambda: _hook
            _m.set_axon_ntff_profile_hook = lambda h: None
            _sys.modules.setdefault("antenv.axon_hooks", _m)
            import concourse.bass_utils as _bu
            _bu.upload_artifacts = lambda tmpdir: f"file://{tmpdir}"
        except Exception:
            trace = False
    return run_gat(cfg, inputs, trace=trace).astype(np.float32)

